# revision 9
# baseline (speedup 1.0000x reference)
"""GRU kernel for Trainium2, 8 NeuronCores, data-parallel over batch.

Strategy (v2)
-------------
reference:  per step t (T=512):
    gi = [h, x_t]; r = sig(gi@Wr+br); z = sig(gi@Wz+bz)
    hh = tanh([h*r, x_t]@Wl+bl); h = (1-z)h + z*hh; out_t = relu(h@Wo+bo)

Decomposition per core (B_local=8 rows, fully transposed domain;
state h^T lives as bf16 [128 part, kc-major 8 chunks x 8 batch cols]):

  Phase 1 (parallel over t): X_g^T = Wx_g^T x^T + b_g, g in {r,z,l}
    (f32r matmuls, N=512) -> DRAM as bf16, layout [H, B_local*T].

  Recurrence (serial over t). Per step:
    - r gate: per out-chunk jc, PSUM initialized with X_r via an
      identity-stationary matmul, then 8 kc matmuls of Wh_r^T h^T
      (bf16 resident weights).  sigmoid straight out of PSUM.
    - z gate: plain matmul accumulation + DVE add of X_z + sigmoid
      (its serial chain hides under the l-gate matmuls).
    - l gate: like r (identity-fold of X_l), rhs = (r*h)^T.
    - update: w = (1-z)*h precomputed during the l window;
      h_new = w + z*tanh(psl)  written directly as bf16 into the
      chunk-major history tile (no separate f32 state, no cast op).
    - every 2nd step, one jc-chunk of the previous block's output
      projection relu(Wo^T h^T + bo) is issued to fill the PE idle
      tail (keeps pairs flowing between the l gate and next r gate).

  PSUM gate pool is 4 deep so step t's gates never wait on step t-1's
  tail reads (the v1 bufs=2 pool serialized exactly that way).
"""
import numpy as np
from contextlib import ExitStack

import concourse.bass as bass
import concourse.tile as tile
from concourse import bacc, mybir
from concourse import bass_utils

B, T_FULL, D, H = 64, 512, 1024, 1024
NCORES = 8
BL = B // NCORES            # 8 batch rows per core
KC = H // 128               # 8 contraction chunks
JC = H // 128               # 8 output chunks
BLK = 16                    # recurrence steps per output-projection block

f32 = mybir.dt.float32
f32r = mybir.dt.float32r
bf16 = mybir.dt.bfloat16
AF = mybir.ActivationFunctionType
ALU = mybir.AluOpType

_CACHE = {}


def build_program(T):
    cols = BL * T           # columns of the transposed activations
    nblk = T // BLK
    assert T % BLK == 0
    CW = BL * KC            # 64: cols of a state tile (kc-major, b minor)

    nc = bacc.Bacc("TRN2", target_bir_lowering=False, debug=False, num_devices=1)

    xT = nc.dram_tensor("xT", (H, cols), f32, kind="ExternalInput").ap()
    wx = {g: nc.dram_tensor(f"wx{g}", (D, H), f32, kind="ExternalInput").ap()
          for g in "rzl"}
    wh = {g: nc.dram_tensor(f"wh{g}", (H, H), f32, kind="ExternalInput").ap()
          for g in "rzl"}
    bias = {g: nc.dram_tensor(f"b{g}", (H, 1), f32, kind="ExternalInput").ap()
            for g in "rzl"}
    wo_d = nc.dram_tensor("wo", (H, H), f32, kind="ExternalInput").ap()
    bo_d = nc.dram_tensor("bo", (H, 1), f32, kind="ExternalInput").ap()
    id_d = nc.dram_tensor("ident", (128, 128), f32, kind="ExternalInput").ap()
    outT = nc.dram_tensor("outT", (128, nblk * JC * BLK * BL), f32,
                          kind="ExternalOutput").ap()

    with tile.TileContext(nc) as tc, ExitStack() as top:
        dram = top.enter_context(tc.tile_pool(name="dram", bufs=1, space="DRAM"))
        xg_d = {g: dram.tile([H, cols], bf16, tag=f"X{g}", name=f"X{g}")
                for g in "rzl"}

        # ---------------- Phase 1: x projections (f32r) ----------------
        with ExitStack() as ctx:
            wp = ctx.enter_context(tc.tile_pool(name="p1w", bufs=1))
            xp = ctx.enter_context(tc.tile_pool(name="p1x", bufs=2))
            pp = ctx.enter_context(tc.tile_pool(name="p1ps", bufs=4, space="PSUM"))
            op = ctx.enter_context(tc.tile_pool(name="p1o", bufs=3))
            bp = ctx.enter_context(tc.tile_pool(name="p1b", bufs=1))

            wx_sb = {}
            bt = {}
            for g in "rzl":
                wx_sb[g] = wp.tile([128, KC * H], f32r, tag=f"wx{g}", name=f"wx{g}sb")
                for kc in range(KC):
                    nc.sync.dma_start(
                        wx_sb[g][:, kc * H:(kc + 1) * H],
                        wx[g][kc * 128:(kc + 1) * 128, :].bitcast(f32r))
                bt[g] = bp.tile([128, JC], f32, tag=f"b{g}", name=f"bt{g}")
                for jc in range(JC):
                    nc.sync.dma_start(bt[g][:, jc:jc + 1],
                                      bias[g][jc * 128:(jc + 1) * 128, :])

            NCB = 512
            for cb in range(cols // NCB):
                xt = xp.tile([128, KC * NCB], f32r, tag="xt")
                for kc in range(KC):
                    nc.sync.dma_start(
                        xt[:, kc * NCB:(kc + 1) * NCB],
                        xT[kc * 128:(kc + 1) * 128,
                           cb * NCB:(cb + 1) * NCB].bitcast(f32r))
                for g in "rzl":
                    for jc in range(JC):
                        ps = pp.tile([128, NCB], f32, tag="ps")
                        for kc in range(KC):
                            nc.tensor.matmul(
                                ps[:],
                                lhsT=wx_sb[g][:, kc * H + jc * 128:
                                              kc * H + (jc + 1) * 128],
                                rhs=xt[:, kc * NCB:(kc + 1) * NCB],
                                start=(kc == 0), stop=(kc == KC - 1))
                        ot = op.tile([128, NCB], bf16, tag="ot")
                        nc.scalar.activation(ot[:], ps[:], AF.Identity,
                                             bias=bt[g][:, jc:jc + 1])
                        nc.sync.dma_start(
                            xg_d[g][jc * 128:(jc + 1) * 128,
                                    cb * NCB:(cb + 1) * NCB], ot[:])

        # Phase-1 writes X* to DRAM via DMA; DRAM-tile RAW deps are not
        # reliably tracked by the scheduler, so fence before consuming.
        tc.strict_bb_all_engine_barrier()

        # ------------- Recurrence + fused output projection -------------
        with ExitStack() as ctx:
            wp = ctx.enter_context(tc.tile_pool(name="rw", bufs=1))
            sg = ctx.enter_context(tc.tile_pool(name="stg", bufs=2))
            xb = ctx.enter_context(tc.tile_pool(name="xblk", bufs=2))
            hi = ctx.enter_context(tc.tile_pool(name="hist", bufs=2))
            el = ctx.enter_context(tc.tile_pool(name="elt", bufs=2))
            pgr = ctx.enter_context(tc.tile_pool(name="psr", bufs=2, space="PSUM"))
            pgz = ctx.enter_context(tc.tile_pool(name="psz", bufs=2, space="PSUM"))
            pgl = ctx.enter_context(tc.tile_pool(name="psl", bufs=2, space="PSUM"))
            p3 = ctx.enter_context(tc.tile_pool(name="ps3", bufs=2, space="PSUM"))
            o3 = ctx.enter_context(tc.tile_pool(name="o3", bufs=3))
            bp = ctx.enter_context(tc.tile_pool(name="rb", bufs=1))

            # resident bf16 weights (staged through f32)
            wh_sb = {}
            for g in "rzl":
                wh_sb[g] = wp.tile([128, KC * H], bf16, tag=f"wh{g}", name=f"wh{g}sb")
                for kc in range(KC):
                    stg = sg.tile([128, H], f32, tag="stg")
                    nc.sync.dma_start(stg[:], wh[g][kc * 128:(kc + 1) * 128, :])
                    nc.vector.tensor_copy(wh_sb[g][:, kc * H:(kc + 1) * H], stg[:])
            wo_sb = wp.tile([128, KC * H], bf16, tag="wo")
            for kc in range(KC):
                stg = sg.tile([128, H], f32, tag="stg")
                nc.sync.dma_start(stg[:], wo_d[kc * 128:(kc + 1) * 128, :])
                nc.vector.tensor_copy(wo_sb[:, kc * H:(kc + 1) * H], stg[:])
            ident = wp.tile([128, 128], bf16, tag="ident")
            stg = sg.tile([128, 128], f32, tag="stg")
            nc.sync.dma_start(stg[:], id_d[:])
            nc.vector.tensor_copy(ident[:], stg[:])
            bo_t = bp.tile([128, JC], f32, tag="bo")
            for jc in range(JC):
                nc.sync.dma_start(bo_t[:, jc:jc + 1],
                                  bo_d[jc * 128:(jc + 1) * 128, :])

            # zero initial state (kc-major layout, one step worth)
            hz = bp.tile([128, CW], bf16, tag="h0")
            nc.vector.memset(hz[:], 0.0)

            hist_prev = None       # previous block's history tile

            def gate_mm(ps, wt, src_slices, xfold=None):
                """Accumulate one gate into ps[:, jc*BL...] for all jc.

                src_slices: per-kc list of [128, BL] bf16 APs (h^T chunks).
                xfold: per-jc [128, BL] bf16 APs added via identity matmul.
                """
                for jc in range(JC):
                    reg = ps[:, jc * BL:(jc + 1) * BL]
                    if xfold is not None:
                        nc.tensor.matmul(reg, lhsT=ident[:],
                                         rhs=xfold[jc], start=True, stop=False)
                    for kc in range(KC):
                        nc.tensor.matmul(
                            reg,
                            lhsT=wt[:, (kc * JC + jc) * 128:
                                    (kc * JC + jc + 1) * 128],
                            rhs=src_slices[kc],
                            start=(xfold is None and kc == 0),
                            stop=(kc == KC - 1))

            def wo_unit(hsrc, bi_out, jc):
                """One jc chunk of the output projection for block bi_out."""
                pso = p3.tile([128, BLK * BL], f32, tag="pso")
                for kc in range(KC):
                    nc.tensor.matmul(
                        pso[:],
                        lhsT=wo_sb[:, (kc * JC + jc) * 128:
                                   (kc * JC + jc + 1) * 128],
                        rhs=hsrc[:, kc * BLK * BL:(kc + 1) * BLK * BL],
                        start=(kc == 0), stop=(kc == KC - 1))
                ou = o3.tile([128, BLK * BL], f32, tag="ou")
                nc.scalar.activation(ou[:], pso[:], AF.Relu,
                                     bias=bo_t[:, jc:jc + 1])
                nc.sync.dma_start(
                    outT[:, (bi_out * JC + jc) * BLK * BL:
                         (bi_out * JC + jc + 1) * BLK * BL], ou[:])

            for bi in range(nblk):
                xblk = {}
                for g in "rzl":
                    xblk[g] = xb.tile([128, JC * BLK * BL], bf16, tag=f"xb{g}",
                                      name=f"xb{g}t")
                    for jc in range(JC):
                        nc.sync.dma_start(
                            xblk[g][:, jc * BLK * BL:(jc + 1) * BLK * BL],
                            xg_d[g][jc * 128:(jc + 1) * 128,
                                    bi * BLK * BL:(bi + 1) * BLK * BL])
                # chunk-major history: col = kc*BLK*BL + t*BL + b
                hist = hi.tile([128, KC * BLK * BL], bf16, tag="hist")
                hview = hist[:].rearrange("p (c t b) -> p c t b", c=KC, t=BLK)

                for dt in range(BLK):
                    if bi == 0 and dt == 0:
                        hsl = [hz[:, kc * BL:(kc + 1) * BL] for kc in range(KC)]
                        hap = hz[:].rearrange("p (c b) -> p c b", c=KC)
                    elif dt == 0:
                        hsl = [hist_prev[:, kc * BLK * BL + (BLK - 1) * BL:
                                         kc * BLK * BL + BLK * BL]
                               for kc in range(KC)]
                        hap = (hist_prev[:]
                               .rearrange("p (c t b) -> p c t b", c=KC, t=BLK)
                               [:, :, BLK - 1, :])
                    else:
                        hsl = [hist[:, kc * BLK * BL + (dt - 1) * BL:
                                    kc * BLK * BL + dt * BL]
                               for kc in range(KC)]
                        hap = hview[:, :, dt - 1, :]

                    def xf(g):
                        return [xblk[g][:, jc * BLK * BL + dt * BL:
                                        jc * BLK * BL + (dt + 1) * BL]
                                for jc in range(JC)]

                    def xap(g):
                        return (xblk[g][:]
                                .rearrange("p (c t b) -> p c t b", c=JC, t=BLK)
                                [:, :, dt, :])

                    c3 = "p (c b) -> p c b"

                    # z gate first: plain accumulation; X_z added on DVE later
                    psz = pgz.tile([128, CW], f32, tag="gz")
                    gate_mm(psz, wh_sb["z"], hsl)
                    # r gate LAST so its PSUM-stop lands at the end of the
                    # contiguous z+r PE run: cross-engine waits only resolve
                    # at run boundaries, so sigmoid-r fires ~2us earlier.
                    psr = pgr.tile([128, CW], f32, tag="gr")
                    gate_mm(psr, wh_sb["r"], hsl, xfold=xf("r"))

                    r = el.tile([128, CW], f32, tag="r")
                    nc.scalar.activation(r[:], psr[:], AF.Sigmoid)
                    rh = el.tile([128, CW], bf16, tag="rh")
                    nc.vector.tensor_mul(rh[:].rearrange(c3, c=KC),
                                         r[:].rearrange(c3, c=KC), hap)

                    # l gate: X_l folded into PSUM, rhs = (r*h)^T
                    psl = pgl.tile([128, CW], f32, tag="gl")
                    gate_mm(psl, wh_sb["l"],
                            [rh[:, kc * BL:(kc + 1) * BL] for kc in range(KC)],
                            xfold=xf("l"))

                    # z post-chain + w = (1-z)*h  (hides under l matmuls)
                    tz = el.tile([128, CW], f32, tag="tz")
                    nc.vector.tensor_add(tz[:].rearrange(c3, c=KC),
                                         psz[:].rearrange(c3, c=KC), xap("z"))
                    z = el.tile([128, CW], f32, tag="z")
                    nc.scalar.activation(z[:], tz[:], AF.Sigmoid)
                    zm1 = el.tile([128, CW], f32, tag="zm1")
                    nc.vector.tensor_scalar(zm1[:], z[:], -1.0, 1.0,
                                            ALU.mult, ALU.add)
                    w = el.tile([128, CW], f32, tag="w")
                    nc.vector.tensor_mul(w[:].rearrange(c3, c=KC),
                                         zm1[:].rearrange(c3, c=KC), hap)

                    # previous block's Wo chunk fills the PE idle tail while
                    # tanh + the h update run on ACT/DVE
                    if bi > 0 and dt % 2 == 0:
                        wo_unit(hist_prev, bi - 1, dt // 2)

                    # tail: hh = tanh(psl); h_new = w + z*hh -> hist (bf16)
                    hh = el.tile([128, CW], f32, tag="hh")
                    nc.scalar.activation(hh[:], psl[:], AF.Tanh)
                    n = el.tile([128, CW], f32, tag="n")
                    nc.vector.tensor_mul(n[:], z[:], hh[:])
                    nc.vector.tensor_add(hview[:, :, dt, :],
                                         w[:].rearrange(c3, c=KC),
                                         n[:].rearrange(c3, c=KC))

                hist_prev = hist

            # output projection for the final block
            for jc in range(JC):
                wo_unit(hist_prev, nblk - 1, jc)

    nc.compile()
    return nc


def get_program(T):
    if T not in _CACHE:
        _CACHE[T] = build_program(T)
    return _CACHE[T]


def make_inmaps(input, Wr, br, Wz, bz, Wl, bl, Wo, bo):
    Tt = input.shape[1]
    cols = BL * Tt
    w_common = {
        "wxr": np.ascontiguousarray(Wr[H:]), "whr": np.ascontiguousarray(Wr[:H]),
        "wxz": np.ascontiguousarray(Wz[H:]), "whz": np.ascontiguousarray(Wz[:H]),
        "wxl": np.ascontiguousarray(Wl[H:]), "whl": np.ascontiguousarray(Wl[:H]),
        "br": np.ascontiguousarray(br.reshape(H, 1)),
        "bz": np.ascontiguousarray(bz.reshape(H, 1)),
        "bl": np.ascontiguousarray(bl.reshape(H, 1)),
        "wo": np.ascontiguousarray(Wo),
        "bo": np.ascontiguousarray(bo.reshape(H, 1)),
        "ident": np.eye(128, dtype=np.float32),
    }
    in_maps = []
    for c in range(NCORES):
        xl = np.asarray(input[c * BL:(c + 1) * BL], dtype=np.float32)
        xTl = np.ascontiguousarray(xl.transpose(2, 1, 0).reshape(H, cols))
        in_maps.append({"xT": xTl, **w_common})
    return in_maps


def kernel(input, Wr, br, Wz, bz, Wl, bl, Wo, bo):
    Tt = input.shape[1]
    prog = get_program(Tt)
    in_maps = make_inmaps(input, Wr, br, Wz, bz, Wl, bl, Wo, bo)
    res = bass_utils.run_bass_kernel_spmd(prog, in_maps,
                                          core_ids=list(range(NCORES)))
    nblk = Tt // BLK
    outs = []
    for c in range(NCORES):
        oT = res.results[c]["outT"]              # [128, nblk*JC*BLK*BL]
        o = oT.reshape(128, nblk, JC, BLK, BL)   # p, bi, j, dt, b
        o = o.transpose(4, 1, 3, 2, 0).reshape(BL, Tt, H)
        outs.append(o)
    return np.ascontiguousarray(np.concatenate(outs, axis=0))


# revision 12
# speedup vs baseline: 1.1032x; 1.1032x over previous
"""GRU kernel for Trainium2, 8 NeuronCores, data-parallel over batch.

Strategy (v4)
-------------
reference:  per step t (T=512):
    gi = [h, x_t]; r = sig(gi@Wr+br); z = sig(gi@Wz+bz)
    hh = tanh([h*r, x_t]@Wl+bl); h = (1-z)h + z*hh; out_t = relu(h@Wo+bo)

Per core (B_local=8 rows, fully transposed domain; state h^T is bf16
[128 part, kc-major 8 chunks x 8 batch cols]):

  All weights ship from the host pre-cast to bf16 and stay resident in
  SBUF.  x^T ships as bf16 and streams per 16-step block.

  The x projections X_g = Wx_g^T x^T + b_g are NOT a separate phase:
  they are computed in-stream, two blocks ahead of the recurrence,
  into an SBUF ring — issued one (gate, jc) unit at a time inside the
  two PE idle windows of each step (the sigmoid_r/rh handoff window
  and the tanh/h-update tail).  This removes the serial up-front phase
  and the DRAM X round-trip entirely.

  Per recurrence step:
    - z gate, then r gate (contiguous PE run).  X_r is folded into
      PSUM via an identity-stationary matmul; sigmoid_r reads PSUM.
    - l gate with X_l identity-folded, rhs = (r*h)^T.
    - update: w = (1-z)*h precomputed during the l window;
      h_new = w + z*tanh(psl) written straight into the chunk-major
      bf16 history tile.
    - idle windows carry: one X-projection unit each, plus one jc
      chunk of the previous block's output projection relu(Wo h + bo)
      every other step.
"""
import numpy as np
import ml_dtypes
from contextlib import ExitStack

import concourse.bass as bass
import concourse.tile as tile
from concourse import bacc, mybir
from concourse import bass_utils

B, T_FULL, D, H = 64, 512, 1024, 1024
NCORES = 8
BL = B // NCORES            # 8 batch rows per core
KC = H // 128               # 8 contraction chunks
JC = H // 128               # 8 output chunks
BLK = 16                    # recurrence steps per output-projection block
CB = BLK * BL               # 128 activation columns per block

f32 = mybir.dt.float32
bf16 = mybir.dt.bfloat16
AF = mybir.ActivationFunctionType
ALU = mybir.AluOpType

_CACHE = {}


def build_program(T):
    cols = BL * T
    nblk = T // BLK
    assert T % BLK == 0
    CW = BL * KC            # 64: cols of a state tile (kc-major, b minor)

    nc = bacc.Bacc("TRN2", target_bir_lowering=False, debug=False, num_devices=1)

    xT = nc.dram_tensor("xT", (H, cols), bf16, kind="ExternalInput").ap()
    wx = {g: nc.dram_tensor(f"wx{g}", (D, H), bf16, kind="ExternalInput").ap()
          for g in "rzl"}
    wh = {g: nc.dram_tensor(f"wh{g}", (H, H), bf16, kind="ExternalInput").ap()
          for g in "rzl"}
    bias = {g: nc.dram_tensor(f"b{g}", (H, 1), f32, kind="ExternalInput").ap()
            for g in "rzl"}
    wo_d = nc.dram_tensor("wo", (H, H), bf16, kind="ExternalInput").ap()
    bo_d = nc.dram_tensor("bo", (H, 1), f32, kind="ExternalInput").ap()
    id_d = nc.dram_tensor("ident", (128, 128), bf16, kind="ExternalInput").ap()
    outT = nc.dram_tensor("outT", (128, nblk * JC * CB), f32,
                          kind="ExternalOutput").ap()

    with tile.TileContext(nc) as tc, ExitStack() as ctx:
        wp = ctx.enter_context(tc.tile_pool(name="rw", bufs=1))
        xtp = ctx.enter_context(tc.tile_pool(name="xtp", bufs=3))
        xsb = ctx.enter_context(tc.tile_pool(name="xsb", bufs=3))
        hi = ctx.enter_context(tc.tile_pool(name="hist", bufs=2))
        el = ctx.enter_context(tc.tile_pool(name="elt", bufs=2))
        pgr = ctx.enter_context(tc.tile_pool(name="psr", bufs=2, space="PSUM"))
        pgz = ctx.enter_context(tc.tile_pool(name="psz", bufs=2, space="PSUM"))
        pgl = ctx.enter_context(tc.tile_pool(name="psl", bufs=2, space="PSUM"))
        p3 = ctx.enter_context(tc.tile_pool(name="ps3", bufs=2, space="PSUM"))
        o3 = ctx.enter_context(tc.tile_pool(name="o3", bufs=3))
        bp = ctx.enter_context(tc.tile_pool(name="rb", bufs=1))

        # resident bf16 weights (shipped pre-cast from the host)
        wx_sb, wh_sb, bt = {}, {}, {}
        for g in "rzl":
            wx_sb[g] = wp.tile([128, KC * H], bf16, tag=f"wx{g}", name=f"wx{g}sb")
            wh_sb[g] = wp.tile([128, KC * H], bf16, tag=f"wh{g}", name=f"wh{g}sb")
            for kc in range(KC):
                nc.sync.dma_start(wx_sb[g][:, kc * H:(kc + 1) * H],
                                  wx[g][kc * 128:(kc + 1) * 128, :])
                nc.sync.dma_start(wh_sb[g][:, kc * H:(kc + 1) * H],
                                  wh[g][kc * 128:(kc + 1) * 128, :])
            bt[g] = bp.tile([128, JC], f32, tag=f"b{g}", name=f"bt{g}")
            for jc in range(JC):
                nc.sync.dma_start(bt[g][:, jc:jc + 1],
                                  bias[g][jc * 128:(jc + 1) * 128, :])
        wo_sb = wp.tile([128, KC * H], bf16, tag="wo")
        for kc in range(KC):
            nc.sync.dma_start(wo_sb[:, kc * H:(kc + 1) * H],
                              wo_d[kc * 128:(kc + 1) * 128, :])
        ident = wp.tile([128, 128], bf16, tag="ident")
        nc.sync.dma_start(ident[:], id_d[:])
        bo_t = bp.tile([128, JC], f32, tag="bo")
        for jc in range(JC):
            nc.sync.dma_start(bo_t[:, jc:jc + 1],
                              bo_d[jc * 128:(jc + 1) * 128, :])

        hz = bp.tile([128, CW], bf16, tag="h0")
        nc.vector.memset(hz[:], 0.0)

        # ---- X-projection machinery (in-stream "phase 1") ----
        xt_blk = {}         # bi -> [128, KC*CB] bf16 input slice
        X_blk = {}          # bi -> {g: [128, JC*CB] bf16}

        def xt_load(bi):
            t_ = xtp.tile([128, KC * CB], bf16, tag="xt")
            for kc in range(KC):
                nc.sync.dma_start(
                    t_[:, kc * CB:(kc + 1) * CB],
                    xT[kc * 128:(kc + 1) * 128, bi * CB:(bi + 1) * CB])
            xt_blk[bi] = t_

        def x_alloc(bi):
            X_blk[bi] = {g: xsb.tile([128, JC * CB], bf16, tag=f"X{g}",
                                     name=f"X{g}b")
                         for g in "rzl"}

        def x_unit(bi, g, jc):
            """One (gate, jc) X-projection unit for block bi (8 MMs N=128)."""
            ps = p3.tile([128, CB], f32, tag="pso")
            xt = xt_blk[bi]
            for kc in range(KC):
                nc.tensor.matmul(
                    ps[:],
                    lhsT=wx_sb[g][:, kc * H + jc * 128:kc * H + (jc + 1) * 128],
                    rhs=xt[:, kc * CB:(kc + 1) * CB],
                    start=(kc == 0), stop=(kc == KC - 1))
            nc.scalar.activation(X_blk[bi][g][:, jc * CB:(jc + 1) * CB],
                                 ps[:], AF.Identity, bias=bt[g][:, jc:jc + 1])

        UNITS = [(g, jc) for g in "rzl" for jc in range(JC)]   # 24 per block

        def wo_unit(hsrc, bi_out, jc):
            pso = p3.tile([128, CB], f32, tag="pso")
            for kc in range(KC):
                nc.tensor.matmul(
                    pso[:],
                    lhsT=wo_sb[:, (kc * JC + jc) * 128:(kc * JC + jc + 1) * 128],
                    rhs=hsrc[:, kc * CB:(kc + 1) * CB],
                    start=(kc == 0), stop=(kc == KC - 1))
            ou = o3.tile([128, CB], f32, tag="ou")
            nc.scalar.activation(ou[:], pso[:], AF.Relu, bias=bo_t[:, jc:jc + 1])
            nc.sync.dma_start(
                outT[:, (bi_out * JC + jc) * CB:(bi_out * JC + jc + 1) * CB],
                ou[:])

        def gate_mm(ps, wt, src_slices, xfold=None):
            for jc in range(JC):
                reg = ps[:, jc * BL:(jc + 1) * BL]
                if xfold is not None:
                    nc.tensor.matmul(reg, lhsT=ident[:], rhs=xfold[jc],
                                     start=True, stop=False)
                for kc in range(KC):
                    nc.tensor.matmul(
                        reg,
                        lhsT=wt[:, (kc * JC + jc) * 128:(kc * JC + jc + 1) * 128],
                        rhs=src_slices[kc],
                        start=(xfold is None and kc == 0),
                        stop=(kc == KC - 1))

        # prime: X projections for blocks 0 and 1 up-front
        for bi in range(min(2, nblk)):
            xt_load(bi)
            x_alloc(bi)
            for g, jc in UNITS:
                x_unit(bi, g, jc)

        hist_prev = None
        for bi in range(nblk):
            if bi + 2 < nblk:
                xt_load(bi + 2)
                x_alloc(bi + 2)
            # X-projection units for block bi+2, spread across this block's
            # steps: 2 units on steps 0-7, 1 unit on steps 8-15.
            queue = list(UNITS) if bi + 2 < nblk else []

            Xb = X_blk[bi]
            hist = hi.tile([128, KC * CB], bf16, tag="hist")
            hview = hist[:].rearrange("p (c t b) -> p c t b", c=KC, t=BLK)

            for dt in range(BLK):
                if bi == 0 and dt == 0:
                    hsl = [hz[:, kc * BL:(kc + 1) * BL] for kc in range(KC)]
                    hap = hz[:].rearrange("p (c b) -> p c b", c=KC)
                elif dt == 0:
                    hsl = [hist_prev[:, kc * CB + (BLK - 1) * BL:
                                     kc * CB + BLK * BL] for kc in range(KC)]
                    hap = (hist_prev[:]
                           .rearrange("p (c t b) -> p c t b", c=KC, t=BLK)
                           [:, :, BLK - 1, :])
                else:
                    hsl = [hist[:, kc * CB + (dt - 1) * BL:kc * CB + dt * BL]
                           for kc in range(KC)]
                    hap = hview[:, :, dt - 1, :]

                def xf(g):
                    return [Xb[g][:, jc * CB + dt * BL:jc * CB + (dt + 1) * BL]
                            for jc in range(JC)]

                def xap(g):
                    return (Xb[g][:]
                            .rearrange("p (c t b) -> p c t b", c=JC, t=BLK)
                            [:, :, dt, :])

                c3 = "p (c b) -> p c b"

                # r gate first, z gate second (measured faster than z-first:
                # the PE semaphore bump covering psr's stop lands mid-z-run,
                # waking sigmoid_r earlier).
                psr = pgr.tile([128, CW], f32, tag="gr")
                gate_mm(psr, wh_sb["r"], hsl, xfold=xf("r"))
                psz = pgz.tile([128, CW], f32, tag="gz")
                gate_mm(psz, wh_sb["z"], hsl)

                r = el.tile([128, CW], f32, tag="r")
                nc.scalar.activation(r[:], psr[:], AF.Sigmoid)
                rh = el.tile([128, CW], bf16, tag="rh")
                nc.vector.tensor_mul(rh[:].rearrange(c3, c=KC),
                                     r[:].rearrange(c3, c=KC), hap)

                # one X unit rides in the sigmoid_r/rh handoff window
                if queue:
                    g_, jc_ = queue.pop(0)
                    x_unit(bi + 2, g_, jc_)

                # l gate: X_l folded into PSUM, rhs = (r*h)^T
                psl = pgl.tile([128, CW], f32, tag="gl")
                gate_mm(psl, wh_sb["l"],
                        [rh[:, kc * BL:(kc + 1) * BL] for kc in range(KC)],
                        xfold=xf("l"))

                # z post-chain + w = (1-z)*h (hides under the l matmuls)
                tz = el.tile([128, CW], f32, tag="tz")
                nc.vector.tensor_add(tz[:].rearrange(c3, c=KC),
                                     psz[:].rearrange(c3, c=KC), xap("z"))
                z = el.tile([128, CW], f32, tag="z")
                nc.scalar.activation(z[:], tz[:], AF.Sigmoid)
                zm1 = el.tile([128, CW], f32, tag="zm1")
                nc.vector.tensor_scalar(zm1[:], z[:], -1.0, 1.0,
                                        ALU.mult, ALU.add)
                w = el.tile([128, CW], f32, tag="w")
                nc.vector.tensor_mul(w[:].rearrange(c3, c=KC),
                                     zm1[:].rearrange(c3, c=KC), hap)

                # tail window: alternate Wo chunk / X unit
                if dt % 2 == 0:
                    if bi > 0:
                        wo_unit(hist_prev, bi - 1, dt // 2)
                elif queue:
                    g_, jc_ = queue.pop(0)
                    x_unit(bi + 2, g_, jc_)

                # tail: hh = tanh(psl); h_new = w + z*hh -> hist (bf16)
                hh = el.tile([128, CW], f32, tag="hh")
                nc.scalar.activation(hh[:], psl[:], AF.Tanh)
                n = el.tile([128, CW], f32, tag="n")
                nc.vector.tensor_mul(n[:], z[:], hh[:])
                nc.vector.tensor_add(hview[:, :, dt, :],
                                     w[:].rearrange(c3, c=KC),
                                     n[:].rearrange(c3, c=KC))

            # issue any leftover units (only when nblk is tiny)
            for g_, jc_ in queue:
                x_unit(bi + 2, g_, jc_)
            X_blk.pop(bi, None)
            xt_blk.pop(bi, None)
            hist_prev = hist

        for jc in range(JC):
            wo_unit(hist_prev, nblk - 1, jc)

    nc.compile()
    return nc


def get_program(T):
    if T not in _CACHE:
        _CACHE[T] = build_program(T)
    return _CACHE[T]


def _bf(a):
    return np.ascontiguousarray(np.asarray(a, dtype=np.float32)).astype(
        ml_dtypes.bfloat16)


def make_inmaps(input, Wr, br, Wz, bz, Wl, bl, Wo, bo):
    Tt = input.shape[1]
    cols = BL * Tt
    w_common = {
        "wxr": _bf(Wr[H:]), "whr": _bf(Wr[:H]),
        "wxz": _bf(Wz[H:]), "whz": _bf(Wz[:H]),
        "wxl": _bf(Wl[H:]), "whl": _bf(Wl[:H]),
        "br": np.ascontiguousarray(br.reshape(H, 1)).astype(np.float32),
        "bz": np.ascontiguousarray(bz.reshape(H, 1)).astype(np.float32),
        "bl": np.ascontiguousarray(bl.reshape(H, 1)).astype(np.float32),
        "wo": _bf(Wo),
        "bo": np.ascontiguousarray(bo.reshape(H, 1)).astype(np.float32),
        "ident": np.eye(128, dtype=np.float32).astype(ml_dtypes.bfloat16),
    }
    in_maps = []
    for c in range(NCORES):
        xl = np.asarray(input[c * BL:(c + 1) * BL], dtype=np.float32)
        xTl = _bf(xl.transpose(2, 1, 0).reshape(H, cols))
        in_maps.append({"xT": xTl, **w_common})
    return in_maps


def kernel(input, Wr, br, Wz, bz, Wl, bl, Wo, bo):
    Tt = input.shape[1]
    prog = get_program(Tt)
    in_maps = make_inmaps(input, Wr, br, Wz, bz, Wl, bl, Wo, bo)
    res = bass_utils.run_bass_kernel_spmd(prog, in_maps,
                                          core_ids=list(range(NCORES)))
    nblk = Tt // BLK
    outs = []
    for c in range(NCORES):
        oT = res.results[c]["outT"]              # [128, nblk*JC*BLK*BL]
        o = oT.reshape(128, nblk, JC, BLK, BL)   # p, bi, j, dt, b
        o = o.transpose(4, 1, 3, 2, 0).reshape(BL, Tt, H)
        outs.append(o)
    return np.ascontiguousarray(np.concatenate(outs, axis=0))


# revision 14
# speedup vs baseline: 1.1035x; 1.0003x over previous
"""GRU kernel for Trainium2, 8 NeuronCores, data-parallel over batch.

Strategy (v4)
-------------
reference:  per step t (T=512):
    gi = [h, x_t]; r = sig(gi@Wr+br); z = sig(gi@Wz+bz)
    hh = tanh([h*r, x_t]@Wl+bl); h = (1-z)h + z*hh; out_t = relu(h@Wo+bo)

Per core (B_local=8 rows, fully transposed domain; state h^T is bf16
[128 part, kc-major 8 chunks x 8 batch cols]):

  All weights ship from the host pre-cast to bf16 and stay resident in
  SBUF.  x^T ships as bf16 and streams per 16-step block.

  The x projections X_g = Wx_g^T x^T + b_g are NOT a separate phase:
  they are computed in-stream, two blocks ahead of the recurrence,
  into an SBUF ring — issued one (gate, jc) unit at a time inside the
  two PE idle windows of each step (the sigmoid_r/rh handoff window
  and the tanh/h-update tail).  This removes the serial up-front phase
  and the DRAM X round-trip entirely.

  Per recurrence step:
    - z gate, then r gate (contiguous PE run).  X_r is folded into
      PSUM via an identity-stationary matmul; sigmoid_r reads PSUM.
    - l gate with X_l identity-folded, rhs = (r*h)^T.
    - update: w = (1-z)*h precomputed during the l window;
      h_new = w + z*tanh(psl) written straight into the chunk-major
      bf16 history tile.
    - idle windows carry: one X-projection unit each, plus one jc
      chunk of the previous block's output projection relu(Wo h + bo)
      every other step.
"""
import numpy as np
import ml_dtypes
from contextlib import ExitStack

import concourse.bass as bass
import concourse.tile as tile
from concourse import bacc, mybir
from concourse import bass_utils

B, T_FULL, D, H = 64, 512, 1024, 1024
NCORES = 8
BL = B // NCORES            # 8 batch rows per core
KC = H // 128               # 8 contraction chunks
JC = H // 128               # 8 output chunks
BLK = 16                    # recurrence steps per output-projection block
CB = BLK * BL               # 128 activation columns per block

f32 = mybir.dt.float32
bf16 = mybir.dt.bfloat16
AF = mybir.ActivationFunctionType
ALU = mybir.AluOpType

_CACHE = {}


def build_program(T):
    cols = BL * T
    nblk = T // BLK
    assert T % BLK == 0
    CW = BL * KC            # 64: cols of a state tile (kc-major, b minor)

    nc = bacc.Bacc("TRN2", target_bir_lowering=False, debug=False, num_devices=1)

    xT = nc.dram_tensor("xT", (H, cols), bf16, kind="ExternalInput").ap()
    wx = {g: nc.dram_tensor(f"wx{g}", (D, H), bf16, kind="ExternalInput").ap()
          for g in "rzl"}
    wh = {g: nc.dram_tensor(f"wh{g}", (H, H), bf16, kind="ExternalInput").ap()
          for g in "rzl"}
    bias = {g: nc.dram_tensor(f"b{g}", (H, 1), f32, kind="ExternalInput").ap()
            for g in "rzl"}
    wo_d = nc.dram_tensor("wo", (H, H), bf16, kind="ExternalInput").ap()
    bo_d = nc.dram_tensor("bo", (H, 1), f32, kind="ExternalInput").ap()
    id_d = nc.dram_tensor("ident", (128, 128), bf16, kind="ExternalInput").ap()
    outT = nc.dram_tensor("outT", (128, nblk * JC * CB), f32,
                          kind="ExternalOutput").ap()

    with tile.TileContext(nc) as tc, ExitStack() as ctx:
        wp = ctx.enter_context(tc.tile_pool(name="rw", bufs=1))
        xtp = ctx.enter_context(tc.tile_pool(name="xtp", bufs=3))
        xsb = ctx.enter_context(tc.tile_pool(name="xsb", bufs=3))
        hi = ctx.enter_context(tc.tile_pool(name="hist", bufs=2))
        el = ctx.enter_context(tc.tile_pool(name="elt", bufs=2))
        rp = ctx.enter_context(tc.tile_pool(name="rrh", bufs=4))
        pgr = ctx.enter_context(tc.tile_pool(name="psr", bufs=2, space="PSUM"))
        pgz = ctx.enter_context(tc.tile_pool(name="psz", bufs=2, space="PSUM"))
        pgl = ctx.enter_context(tc.tile_pool(name="psl", bufs=2, space="PSUM"))
        p3 = ctx.enter_context(tc.tile_pool(name="ps3", bufs=2, space="PSUM"))
        o3 = ctx.enter_context(tc.tile_pool(name="o3", bufs=3))
        bp = ctx.enter_context(tc.tile_pool(name="rb", bufs=1))

        # resident bf16 weights (shipped pre-cast from the host)
        wx_sb, wh_sb, bt = {}, {}, {}
        for g in "rzl":
            wx_sb[g] = wp.tile([128, KC * H], bf16, tag=f"wx{g}", name=f"wx{g}sb")
            wh_sb[g] = wp.tile([128, KC * H], bf16, tag=f"wh{g}", name=f"wh{g}sb")
            for kc in range(KC):
                nc.sync.dma_start(wx_sb[g][:, kc * H:(kc + 1) * H],
                                  wx[g][kc * 128:(kc + 1) * 128, :])
                nc.sync.dma_start(wh_sb[g][:, kc * H:(kc + 1) * H],
                                  wh[g][kc * 128:(kc + 1) * 128, :])
            bt[g] = bp.tile([128, JC], f32, tag=f"b{g}", name=f"bt{g}")
            for jc in range(JC):
                nc.sync.dma_start(bt[g][:, jc:jc + 1],
                                  bias[g][jc * 128:(jc + 1) * 128, :])
        wo_sb = wp.tile([128, KC * H], bf16, tag="wo")
        for kc in range(KC):
            nc.sync.dma_start(wo_sb[:, kc * H:(kc + 1) * H],
                              wo_d[kc * 128:(kc + 1) * 128, :])
        ident = wp.tile([128, 128], bf16, tag="ident")
        nc.sync.dma_start(ident[:], id_d[:])
        bo_t = bp.tile([128, JC], f32, tag="bo")
        for jc in range(JC):
            nc.sync.dma_start(bo_t[:, jc:jc + 1],
                              bo_d[jc * 128:(jc + 1) * 128, :])

        hz = bp.tile([128, CW], bf16, tag="h0")
        nc.vector.memset(hz[:], 0.0)

        # ---- X-projection machinery (in-stream "phase 1") ----
        xt_blk = {}         # bi -> [128, KC*CB] bf16 input slice
        X_blk = {}          # bi -> {g: [128, JC*CB] bf16}

        def xt_load(bi):
            t_ = xtp.tile([128, KC * CB], bf16, tag="xt")
            for kc in range(KC):
                nc.sync.dma_start(
                    t_[:, kc * CB:(kc + 1) * CB],
                    xT[kc * 128:(kc + 1) * 128, bi * CB:(bi + 1) * CB])
            xt_blk[bi] = t_

        def x_alloc(bi):
            X_blk[bi] = {g: xsb.tile([128, JC * CB], bf16, tag=f"X{g}",
                                     name=f"X{g}b")
                         for g in "rzl"}

        def x_unit(bi, g, jc):
            """One (gate, jc) X-projection unit for block bi (8 MMs N=128)."""
            ps = p3.tile([128, CB], f32, tag="pso")
            xt = xt_blk[bi]
            for kc in range(KC):
                nc.tensor.matmul(
                    ps[:],
                    lhsT=wx_sb[g][:, kc * H + jc * 128:kc * H + (jc + 1) * 128],
                    rhs=xt[:, kc * CB:(kc + 1) * CB],
                    start=(kc == 0), stop=(kc == KC - 1))
            nc.scalar.activation(X_blk[bi][g][:, jc * CB:(jc + 1) * CB],
                                 ps[:], AF.Identity, bias=bt[g][:, jc:jc + 1])

        UNITS = [(g, jc) for g in "rzl" for jc in range(JC)]   # 24 per block

        def wo_unit(hsrc, bi_out, jc):
            pso = p3.tile([128, CB], f32, tag="pso")
            for kc in range(KC):
                nc.tensor.matmul(
                    pso[:],
                    lhsT=wo_sb[:, (kc * JC + jc) * 128:(kc * JC + jc + 1) * 128],
                    rhs=hsrc[:, kc * CB:(kc + 1) * CB],
                    start=(kc == 0), stop=(kc == KC - 1))
            ou = o3.tile([128, CB], f32, tag="ou")
            nc.scalar.activation(ou[:], pso[:], AF.Relu, bias=bo_t[:, jc:jc + 1])
            nc.sync.dma_start(
                outT[:, (bi_out * JC + jc) * CB:(bi_out * JC + jc + 1) * CB],
                ou[:])

        def gate_mm(ps, wt, src_slices, xfold=None):
            for jc in range(JC):
                reg = ps[:, jc * BL:(jc + 1) * BL]
                if xfold is not None:
                    nc.tensor.matmul(reg, lhsT=ident[:], rhs=xfold[jc],
                                     start=True, stop=False)
                for kc in range(KC):
                    nc.tensor.matmul(
                        reg,
                        lhsT=wt[:, (kc * JC + jc) * 128:(kc * JC + jc + 1) * 128],
                        rhs=src_slices[kc],
                        start=(xfold is None and kc == 0),
                        stop=(kc == KC - 1))

        # prime: X projections for blocks 0 and 1 up-front
        for bi in range(min(2, nblk)):
            xt_load(bi)
            x_alloc(bi)
            for g, jc in UNITS:
                x_unit(bi, g, jc)

        hist_prev = None
        for bi in range(nblk):
            if bi + 2 < nblk:
                xt_load(bi + 2)
                x_alloc(bi + 2)
            # X-projection units for block bi+2, spread across this block's
            # steps: 2 units on steps 0-7, 1 unit on steps 8-15.
            queue = list(UNITS) if bi + 2 < nblk else []

            Xb = X_blk[bi]
            hist = hi.tile([128, KC * CB], bf16, tag="hist")
            hview = hist[:].rearrange("p (c t b) -> p c t b", c=KC, t=BLK)

            for dt in range(BLK):
                if bi == 0 and dt == 0:
                    hsl = [hz[:, kc * BL:(kc + 1) * BL] for kc in range(KC)]
                    hap = hz[:].rearrange("p (c b) -> p c b", c=KC)
                elif dt == 0:
                    hsl = [hist_prev[:, kc * CB + (BLK - 1) * BL:
                                     kc * CB + BLK * BL] for kc in range(KC)]
                    hap = (hist_prev[:]
                           .rearrange("p (c t b) -> p c t b", c=KC, t=BLK)
                           [:, :, BLK - 1, :])
                else:
                    hsl = [hist[:, kc * CB + (dt - 1) * BL:kc * CB + dt * BL]
                           for kc in range(KC)]
                    hap = hview[:, :, dt - 1, :]

                def xf(g):
                    return [Xb[g][:, jc * CB + dt * BL:jc * CB + (dt + 1) * BL]
                            for jc in range(JC)]

                def xap(g):
                    return (Xb[g][:]
                            .rearrange("p (c t b) -> p c t b", c=JC, t=BLK)
                            [:, :, dt, :])

                c3 = "p (c b) -> p c b"

                # r gate first, z gate second (measured faster than z-first:
                # the PE semaphore bump covering psr's stop lands mid-z-run,
                # waking sigmoid_r earlier).
                psr = pgr.tile([128, CW], f32, tag="gr")
                gate_mm(psr, wh_sb["r"], hsl, xfold=xf("r"))
                psz = pgz.tile([128, CW], f32, tag="gz")
                gate_mm(psz, wh_sb["z"], hsl)

                r = rp.tile([128, CW], f32, tag="r")
                nc.scalar.activation(r[:], psr[:], AF.Sigmoid)
                rh = rp.tile([128, CW], bf16, tag="rh")
                nc.vector.tensor_mul(rh[:].rearrange(c3, c=KC),
                                     r[:].rearrange(c3, c=KC), hap)

                # one X unit rides in the sigmoid_r/rh handoff window
                if queue:
                    g_, jc_ = queue.pop(0)
                    x_unit(bi + 2, g_, jc_)

                # l gate: X_l folded into PSUM, rhs = (r*h)^T
                psl = pgl.tile([128, CW], f32, tag="gl")
                gate_mm(psl, wh_sb["l"],
                        [rh[:, kc * BL:(kc + 1) * BL] for kc in range(KC)],
                        xfold=xf("l"))

                # z post-chain + w = (1-z)*h (hides under the l matmuls)
                tz = el.tile([128, CW], f32, tag="tz")
                nc.vector.tensor_add(tz[:].rearrange(c3, c=KC),
                                     psz[:].rearrange(c3, c=KC), xap("z"))
                z = el.tile([128, CW], f32, tag="z")
                nc.scalar.activation(z[:], tz[:], AF.Sigmoid)
                zm1 = el.tile([128, CW], f32, tag="zm1")
                nc.vector.tensor_scalar(zm1[:], z[:], -1.0, 1.0,
                                        ALU.mult, ALU.add)
                w = el.tile([128, CW], f32, tag="w")
                nc.vector.tensor_mul(w[:].rearrange(c3, c=KC),
                                     zm1[:].rearrange(c3, c=KC), hap)

                # tail window: alternate Wo chunk / X unit
                if dt % 2 == 0:
                    if bi > 0:
                        wo_unit(hist_prev, bi - 1, dt // 2)
                elif queue:
                    g_, jc_ = queue.pop(0)
                    x_unit(bi + 2, g_, jc_)

                # tail: hh = tanh(psl); h_new = w + z*hh -> hist (bf16)
                hh = el.tile([128, CW], f32, tag="hh")
                nc.scalar.activation(hh[:], psl[:], AF.Tanh)
                n = el.tile([128, CW], f32, tag="n")
                nc.vector.tensor_mul(n[:], z[:], hh[:])
                nc.vector.tensor_add(hview[:, :, dt, :],
                                     w[:].rearrange(c3, c=KC),
                                     n[:].rearrange(c3, c=KC))

            # issue any leftover units (only when nblk is tiny)
            for g_, jc_ in queue:
                x_unit(bi + 2, g_, jc_)
            X_blk.pop(bi, None)
            xt_blk.pop(bi, None)
            hist_prev = hist

        for jc in range(JC):
            wo_unit(hist_prev, nblk - 1, jc)

    nc.compile()
    return nc


def get_program(T):
    if T not in _CACHE:
        _CACHE[T] = build_program(T)
    return _CACHE[T]


def _bf(a):
    return np.ascontiguousarray(np.asarray(a, dtype=np.float32)).astype(
        ml_dtypes.bfloat16)


def make_inmaps(input, Wr, br, Wz, bz, Wl, bl, Wo, bo):
    Tt = input.shape[1]
    cols = BL * Tt
    w_common = {
        "wxr": _bf(Wr[H:]), "whr": _bf(Wr[:H]),
        "wxz": _bf(Wz[H:]), "whz": _bf(Wz[:H]),
        "wxl": _bf(Wl[H:]), "whl": _bf(Wl[:H]),
        "br": np.ascontiguousarray(br.reshape(H, 1)).astype(np.float32),
        "bz": np.ascontiguousarray(bz.reshape(H, 1)).astype(np.float32),
        "bl": np.ascontiguousarray(bl.reshape(H, 1)).astype(np.float32),
        "wo": _bf(Wo),
        "bo": np.ascontiguousarray(bo.reshape(H, 1)).astype(np.float32),
        "ident": np.eye(128, dtype=np.float32).astype(ml_dtypes.bfloat16),
    }
    in_maps = []
    for c in range(NCORES):
        xl = np.asarray(input[c * BL:(c + 1) * BL], dtype=np.float32)
        xTl = _bf(xl.transpose(2, 1, 0).reshape(H, cols))
        in_maps.append({"xT": xTl, **w_common})
    return in_maps


def kernel(input, Wr, br, Wz, bz, Wl, bl, Wo, bo):
    Tt = input.shape[1]
    prog = get_program(Tt)
    in_maps = make_inmaps(input, Wr, br, Wz, bz, Wl, bl, Wo, bo)
    res = bass_utils.run_bass_kernel_spmd(prog, in_maps,
                                          core_ids=list(range(NCORES)))
    nblk = Tt // BLK
    outs = []
    for c in range(NCORES):
        oT = res.results[c]["outT"]              # [128, nblk*JC*BLK*BL]
        o = oT.reshape(128, nblk, JC, BLK, BL)   # p, bi, j, dt, b
        o = o.transpose(4, 1, 3, 2, 0).reshape(BL, Tt, H)
        outs.append(o)
    return np.ascontiguousarray(np.concatenate(outs, axis=0))


# revision 17
# speedup vs baseline: 1.1569x; 1.0484x over previous
"""GRU kernel for Trainium2, 8 NeuronCores, data-parallel over batch.

Strategy (v4)
-------------
reference:  per step t (T=512):
    gi = [h, x_t]; r = sig(gi@Wr+br); z = sig(gi@Wz+bz)
    hh = tanh([h*r, x_t]@Wl+bl); h = (1-z)h + z*hh; out_t = relu(h@Wo+bo)

Per core (B_local=8 rows, fully transposed domain; state h^T is bf16
[128 part, kc-major 8 chunks x 8 batch cols]):

  All weights ship from the host pre-cast to bf16 and stay resident in
  SBUF.  x^T ships as bf16 and streams per 16-step block.

  The x projections X_g = Wx_g^T x^T + b_g are NOT a separate phase:
  they are computed in-stream, two blocks ahead of the recurrence,
  into an SBUF ring — issued one (gate, jc) unit at a time inside the
  two PE idle windows of each step (the sigmoid_r/rh handoff window
  and the tanh/h-update tail).  This removes the serial up-front phase
  and the DRAM X round-trip entirely.

  Per recurrence step:
    - z gate, then r gate (contiguous PE run).  X_r is folded into
      PSUM via an identity-stationary matmul; sigmoid_r reads PSUM.
    - l gate with X_l identity-folded, rhs = (r*h)^T.
    - update: w = (1-z)*h precomputed during the l window;
      h_new = w + z*tanh(psl) written straight into the chunk-major
      bf16 history tile.
    - idle windows carry: one X-projection unit each, plus one jc
      chunk of the previous block's output projection relu(Wo h + bo)
      every other step.
"""
import numpy as np
import ml_dtypes
from contextlib import ExitStack

import concourse.bass as bass
import concourse.tile as tile
from concourse import bacc, mybir
from concourse import bass_utils

B, T_FULL, D, H = 64, 512, 1024, 1024
NCORES = 8
BL = B // NCORES            # 8 batch rows per core
KC = H // 128               # 8 contraction chunks
JC = H // 128               # 8 output chunks
BLK = 16                    # recurrence steps per output-projection block
CB = BLK * BL               # 128 activation columns per block

f32 = mybir.dt.float32
bf16 = mybir.dt.bfloat16
AF = mybir.ActivationFunctionType
ALU = mybir.AluOpType

_CACHE = {}


def build_program(T):
    cols = BL * T
    nblk = T // BLK
    assert T % BLK == 0
    CW = BL * KC            # 64: cols of a state tile (kc-major, b minor)

    nc = bacc.Bacc("TRN2", target_bir_lowering=False, debug=False, num_devices=1)

    xT = nc.dram_tensor("xT", (H, cols), bf16, kind="ExternalInput").ap()
    wx = {g: nc.dram_tensor(f"wx{g}", (D, H), bf16, kind="ExternalInput").ap()
          for g in "rzl"}
    wh = {g: nc.dram_tensor(f"wh{g}", (H, H), bf16, kind="ExternalInput").ap()
          for g in "rzl"}
    bias = {g: nc.dram_tensor(f"b{g}", (H, 1), f32, kind="ExternalInput").ap()
            for g in "rzl"}
    wo_d = nc.dram_tensor("wo", (H, H), bf16, kind="ExternalInput").ap()
    bo_d = nc.dram_tensor("bo", (H, 1), f32, kind="ExternalInput").ap()
    id_d = nc.dram_tensor("ident", (128, 128), bf16, kind="ExternalInput").ap()
    outT = nc.dram_tensor("outT", (128, nblk * JC * CB), f32,
                          kind="ExternalOutput").ap()

    with tile.TileContext(nc) as tc, ExitStack() as ctx:
        wp = ctx.enter_context(tc.tile_pool(name="rw", bufs=1))
        xtp = ctx.enter_context(tc.tile_pool(name="xtp", bufs=3))
        xsb = ctx.enter_context(tc.tile_pool(name="xsb", bufs=3))
        hi = ctx.enter_context(tc.tile_pool(name="hist", bufs=2))
        el = ctx.enter_context(tc.tile_pool(name="elt", bufs=2))
        rp = ctx.enter_context(tc.tile_pool(name="rrh", bufs=4))
        pgr = ctx.enter_context(tc.tile_pool(name="psr", bufs=2, space="PSUM"))
        pgz = ctx.enter_context(tc.tile_pool(name="psz", bufs=2, space="PSUM"))
        pgl = ctx.enter_context(tc.tile_pool(name="psl", bufs=2, space="PSUM"))
        p3 = ctx.enter_context(tc.tile_pool(name="ps3", bufs=2, space="PSUM"))
        o3 = ctx.enter_context(tc.tile_pool(name="o3", bufs=3))
        bp = ctx.enter_context(tc.tile_pool(name="rb", bufs=1))

        # resident bf16 weights (shipped pre-cast from the host)
        wx_sb, wh_sb, bt = {}, {}, {}
        for g in "rzl":
            wx_sb[g] = wp.tile([128, KC * H], bf16, tag=f"wx{g}", name=f"wx{g}sb")
            wh_sb[g] = wp.tile([128, KC * H], bf16, tag=f"wh{g}", name=f"wh{g}sb")
            for kc in range(KC):
                nc.sync.dma_start(wx_sb[g][:, kc * H:(kc + 1) * H],
                                  wx[g][kc * 128:(kc + 1) * 128, :])
                nc.sync.dma_start(wh_sb[g][:, kc * H:(kc + 1) * H],
                                  wh[g][kc * 128:(kc + 1) * 128, :])
            bt[g] = bp.tile([128, JC], f32, tag=f"b{g}", name=f"bt{g}")
            for jc in range(JC):
                nc.sync.dma_start(bt[g][:, jc:jc + 1],
                                  bias[g][jc * 128:(jc + 1) * 128, :])
        wo_sb = wp.tile([128, KC * H], bf16, tag="wo")
        for kc in range(KC):
            nc.sync.dma_start(wo_sb[:, kc * H:(kc + 1) * H],
                              wo_d[kc * 128:(kc + 1) * 128, :])
        ident = wp.tile([128, 128], bf16, tag="ident")
        nc.sync.dma_start(ident[:], id_d[:])
        bo_t = bp.tile([128, JC], f32, tag="bo")
        for jc in range(JC):
            nc.sync.dma_start(bo_t[:, jc:jc + 1],
                              bo_d[jc * 128:(jc + 1) * 128, :])

        hz = bp.tile([128, CW], bf16, tag="h0")
        nc.vector.memset(hz[:], 0.0)

        # ---- X-projection machinery (in-stream "phase 1") ----
        xt_blk = {}         # bi -> [128, KC*CB] bf16 input slice
        X_blk = {}          # bi -> {g: [128, JC*CB] bf16}

        def xt_load(bi):
            t_ = xtp.tile([128, KC * CB], bf16, tag="xt")
            for kc in range(KC):
                nc.sync.dma_start(
                    t_[:, kc * CB:(kc + 1) * CB],
                    xT[kc * 128:(kc + 1) * 128, bi * CB:(bi + 1) * CB])
            xt_blk[bi] = t_

        def x_alloc(bi):
            X_blk[bi] = {g: xsb.tile([128, JC * CB], bf16, tag=f"X{g}",
                                     name=f"X{g}b")
                         for g in "rzl"}

        def x_unit(bi, g, jc):
            """One (gate, jc) X-projection unit for block bi (8 MMs N=128)."""
            ps = p3.tile([128, CB], f32, tag="pso")
            xt = xt_blk[bi]
            for kc in range(KC):
                nc.tensor.matmul(
                    ps[:],
                    lhsT=wx_sb[g][:, kc * H + jc * 128:kc * H + (jc + 1) * 128],
                    rhs=xt[:, kc * CB:(kc + 1) * CB],
                    start=(kc == 0), stop=(kc == KC - 1))
            nc.scalar.activation(X_blk[bi][g][:, jc * CB:(jc + 1) * CB],
                                 ps[:], AF.Identity, bias=bt[g][:, jc:jc + 1])

        UNITS = [(g, jc) for g in "rzl" for jc in range(JC)]   # 24 per block

        def wo_unit(hsrc, bi_out, jc):
            pso = p3.tile([128, CB], f32, tag="pso")
            for kc in range(KC):
                nc.tensor.matmul(
                    pso[:],
                    lhsT=wo_sb[:, (kc * JC + jc) * 128:(kc * JC + jc + 1) * 128],
                    rhs=hsrc[:, kc * CB:(kc + 1) * CB],
                    start=(kc == 0), stop=(kc == KC - 1))
            ou = o3.tile([128, CB], f32, tag="ou")
            nc.scalar.activation(ou[:], pso[:], AF.Relu, bias=bo_t[:, jc:jc + 1])
            nc.sync.dma_start(
                outT[:, (bi_out * JC + jc) * CB:(bi_out * JC + jc + 1) * CB],
                ou[:])

        def gate_mm(ps, wt, src_slices, xfold=None):
            for jc in range(JC):
                reg = ps[:, jc * BL:(jc + 1) * BL]
                if xfold is not None:
                    nc.tensor.matmul(reg, lhsT=ident[:], rhs=xfold[jc],
                                     start=True, stop=False)
                for kc in range(KC):
                    nc.tensor.matmul(
                        reg,
                        lhsT=wt[:, (kc * JC + jc) * 128:(kc * JC + jc + 1) * 128],
                        rhs=src_slices[kc],
                        start=(xfold is None and kc == 0),
                        stop=(kc == KC - 1))

        # prime: X projections for blocks 0 and 1 up-front
        for bi in range(min(2, nblk)):
            xt_load(bi)
            x_alloc(bi)
            for g, jc in UNITS:
                x_unit(bi, g, jc)

        hist_prev = None
        for bi in range(nblk):
            if bi + 2 < nblk:
                xt_load(bi + 2)
                x_alloc(bi + 2)
            # X-projection units for block bi+2, spread across this block's
            # steps: 2 units on steps 0-7, 1 unit on steps 8-15.
            queue = list(UNITS) if bi + 2 < nblk else []

            Xb = X_blk[bi]
            hist = hi.tile([128, KC * CB], bf16, tag="hist")
            hview = hist[:].rearrange("p (c t b) -> p c t b", c=KC, t=BLK)

            for dt in range(BLK):
                if bi == 0 and dt == 0:
                    hsl = [hz[:, kc * BL:(kc + 1) * BL] for kc in range(KC)]
                    hap = hz[:].rearrange("p (c b) -> p c b", c=KC)
                elif dt == 0:
                    hsl = [hist_prev[:, kc * CB + (BLK - 1) * BL:
                                     kc * CB + BLK * BL] for kc in range(KC)]
                    hap = (hist_prev[:]
                           .rearrange("p (c t b) -> p c t b", c=KC, t=BLK)
                           [:, :, BLK - 1, :])
                else:
                    hsl = [hist[:, kc * CB + (dt - 1) * BL:kc * CB + dt * BL]
                           for kc in range(KC)]
                    hap = hview[:, :, dt - 1, :]

                def xf(g):
                    return [Xb[g][:, jc * CB + dt * BL:jc * CB + (dt + 1) * BL]
                            for jc in range(JC)]

                def xap(g):
                    return (Xb[g][:]
                            .rearrange("p (c t b) -> p c t b", c=JC, t=BLK)
                            [:, :, dt, :])

                c3 = "p (c b) -> p c b"

                # r gate first, z gate second (measured faster than z-first:
                # the PE semaphore bump covering psr's stop lands mid-z-run,
                # waking sigmoid_r earlier).
                psr = pgr.tile([128, CW], f32, tag="gr")
                gate_mm(psr, wh_sb["r"], hsl, xfold=xf("r"))
                psz = pgz.tile([128, CW], f32, tag="gz")
                gate_mm(psz, wh_sb["z"], hsl)

                r = rp.tile([128, CW], f32, tag="r")
                nc.scalar.activation(r[:], psr[:], AF.Sigmoid)
                rh = rp.tile([128, CW], bf16, tag="rh")
                nc.vector.tensor_mul(rh[:].rearrange(c3, c=KC),
                                     r[:].rearrange(c3, c=KC), hap)
                # scheduler fence: keep rh's engine wait from being merged
                # with tz's (whose z-PSUM threshold would delay rh, and
                # with it the whole l gate, to the end of the z matmuls)
                tc.no_sync_barrier()

                # one X unit rides in the sigmoid_r/rh handoff window
                if queue:
                    g_, jc_ = queue.pop(0)
                    x_unit(bi + 2, g_, jc_)

                # l gate: X_l folded into PSUM, rhs = (r*h)^T
                psl = pgl.tile([128, CW], f32, tag="gl")
                gate_mm(psl, wh_sb["l"],
                        [rh[:, kc * BL:(kc + 1) * BL] for kc in range(KC)],
                        xfold=xf("l"))

                # z post-chain + w = (1-z)*h (hides under the l matmuls)
                tz = el.tile([128, CW], f32, tag="tz")
                nc.vector.tensor_add(tz[:].rearrange(c3, c=KC),
                                     psz[:].rearrange(c3, c=KC), xap("z"))
                z = el.tile([128, CW], f32, tag="z")
                nc.scalar.activation(z[:], tz[:], AF.Sigmoid)
                zm1 = el.tile([128, CW], f32, tag="zm1")
                nc.vector.tensor_scalar(zm1[:], z[:], -1.0, 1.0,
                                        ALU.mult, ALU.add)
                w = el.tile([128, CW], f32, tag="w")
                nc.vector.tensor_mul(w[:].rearrange(c3, c=KC),
                                     zm1[:].rearrange(c3, c=KC), hap)

                # tail window: alternate Wo chunk / X unit
                if dt % 2 == 0:
                    if bi > 0:
                        wo_unit(hist_prev, bi - 1, dt // 2)
                elif queue:
                    g_, jc_ = queue.pop(0)
                    x_unit(bi + 2, g_, jc_)

                # tail: hh = tanh(psl); h_new = w + z*hh -> hist (bf16)
                hh = el.tile([128, CW], f32, tag="hh")
                nc.scalar.activation(hh[:], psl[:], AF.Tanh)
                n = el.tile([128, CW], f32, tag="n")
                nc.vector.tensor_mul(n[:], z[:], hh[:])
                nc.vector.tensor_add(hview[:, :, dt, :],
                                     w[:].rearrange(c3, c=KC),
                                     n[:].rearrange(c3, c=KC))

            # issue any leftover units (only when nblk is tiny)
            for g_, jc_ in queue:
                x_unit(bi + 2, g_, jc_)
            X_blk.pop(bi, None)
            xt_blk.pop(bi, None)
            hist_prev = hist

        for jc in range(JC):
            wo_unit(hist_prev, nblk - 1, jc)

    nc.compile()
    return nc


def get_program(T):
    if T not in _CACHE:
        _CACHE[T] = build_program(T)
    return _CACHE[T]


def _bf(a):
    return np.ascontiguousarray(np.asarray(a, dtype=np.float32)).astype(
        ml_dtypes.bfloat16)


def make_inmaps(input, Wr, br, Wz, bz, Wl, bl, Wo, bo):
    Tt = input.shape[1]
    cols = BL * Tt
    w_common = {
        "wxr": _bf(Wr[H:]), "whr": _bf(Wr[:H]),
        "wxz": _bf(Wz[H:]), "whz": _bf(Wz[:H]),
        "wxl": _bf(Wl[H:]), "whl": _bf(Wl[:H]),
        "br": np.ascontiguousarray(br.reshape(H, 1)).astype(np.float32),
        "bz": np.ascontiguousarray(bz.reshape(H, 1)).astype(np.float32),
        "bl": np.ascontiguousarray(bl.reshape(H, 1)).astype(np.float32),
        "wo": _bf(Wo),
        "bo": np.ascontiguousarray(bo.reshape(H, 1)).astype(np.float32),
        "ident": np.eye(128, dtype=np.float32).astype(ml_dtypes.bfloat16),
    }
    in_maps = []
    for c in range(NCORES):
        xl = np.asarray(input[c * BL:(c + 1) * BL], dtype=np.float32)
        xTl = _bf(xl.transpose(2, 1, 0).reshape(H, cols))
        in_maps.append({"xT": xTl, **w_common})
    return in_maps


def kernel(input, Wr, br, Wz, bz, Wl, bl, Wo, bo):
    Tt = input.shape[1]
    prog = get_program(Tt)
    in_maps = make_inmaps(input, Wr, br, Wz, bz, Wl, bl, Wo, bo)
    res = bass_utils.run_bass_kernel_spmd(prog, in_maps,
                                          core_ids=list(range(NCORES)))
    nblk = Tt // BLK
    outs = []
    for c in range(NCORES):
        oT = res.results[c]["outT"]              # [128, nblk*JC*BLK*BL]
        o = oT.reshape(128, nblk, JC, BLK, BL)   # p, bi, j, dt, b
        o = o.transpose(4, 1, 3, 2, 0).reshape(BL, Tt, H)
        outs.append(o)
    return np.ascontiguousarray(np.concatenate(outs, axis=0))


# revision 18
# speedup vs baseline: 1.1571x; 1.0002x over previous
"""GRU kernel for Trainium2, 8 NeuronCores, data-parallel over batch.

Strategy (v4)
-------------
reference:  per step t (T=512):
    gi = [h, x_t]; r = sig(gi@Wr+br); z = sig(gi@Wz+bz)
    hh = tanh([h*r, x_t]@Wl+bl); h = (1-z)h + z*hh; out_t = relu(h@Wo+bo)

Per core (B_local=8 rows, fully transposed domain; state h^T is bf16
[128 part, kc-major 8 chunks x 8 batch cols]):

  All weights ship from the host pre-cast to bf16 and stay resident in
  SBUF.  x^T ships as bf16 and streams per 16-step block.

  The x projections X_g = Wx_g^T x^T + b_g are NOT a separate phase:
  they are computed in-stream, two blocks ahead of the recurrence,
  into an SBUF ring — issued one (gate, jc) unit at a time inside the
  two PE idle windows of each step (the sigmoid_r/rh handoff window
  and the tanh/h-update tail).  This removes the serial up-front phase
  and the DRAM X round-trip entirely.

  Per recurrence step:
    - z gate, then r gate (contiguous PE run).  X_r is folded into
      PSUM via an identity-stationary matmul; sigmoid_r reads PSUM.
    - l gate with X_l identity-folded, rhs = (r*h)^T.
    - update: w = (1-z)*h precomputed during the l window;
      h_new = w + z*tanh(psl) written straight into the chunk-major
      bf16 history tile.
    - idle windows carry: one X-projection unit each, plus one jc
      chunk of the previous block's output projection relu(Wo h + bo)
      every other step.
"""
import numpy as np
import ml_dtypes
from contextlib import ExitStack

import concourse.bass as bass
import concourse.tile as tile
from concourse import bacc, mybir
from concourse import bass_utils

B, T_FULL, D, H = 64, 512, 1024, 1024
NCORES = 8
BL = B // NCORES            # 8 batch rows per core
KC = H // 128               # 8 contraction chunks
JC = H // 128               # 8 output chunks
BLK = 16                    # recurrence steps per output-projection block
CB = BLK * BL               # 128 activation columns per block

f32 = mybir.dt.float32
bf16 = mybir.dt.bfloat16
AF = mybir.ActivationFunctionType
ALU = mybir.AluOpType

_CACHE = {}


def build_program(T):
    cols = BL * T
    nblk = T // BLK
    assert T % BLK == 0
    CW = BL * KC            # 64: cols of a state tile (kc-major, b minor)

    nc = bacc.Bacc("TRN2", target_bir_lowering=False, debug=False, num_devices=1)

    xT = nc.dram_tensor("xT", (H, cols), bf16, kind="ExternalInput").ap()
    wx = {g: nc.dram_tensor(f"wx{g}", (D, H), bf16, kind="ExternalInput").ap()
          for g in "rzl"}
    wh = {g: nc.dram_tensor(f"wh{g}", (H, H), bf16, kind="ExternalInput").ap()
          for g in "rzl"}
    bias = {g: nc.dram_tensor(f"b{g}", (H, 1), f32, kind="ExternalInput").ap()
            for g in "rzl"}
    wo_d = nc.dram_tensor("wo", (H, H), bf16, kind="ExternalInput").ap()
    bo_d = nc.dram_tensor("bo", (H, 1), f32, kind="ExternalInput").ap()
    id_d = nc.dram_tensor("ident", (128, 128), bf16, kind="ExternalInput").ap()
    outT = nc.dram_tensor("outT", (128, nblk * JC * CB), f32,
                          kind="ExternalOutput").ap()

    with tile.TileContext(nc) as tc, ExitStack() as ctx:
        wp = ctx.enter_context(tc.tile_pool(name="rw", bufs=1))
        xtp = ctx.enter_context(tc.tile_pool(name="xtp", bufs=3))
        xsb = ctx.enter_context(tc.tile_pool(name="xsb", bufs=3))
        hi = ctx.enter_context(tc.tile_pool(name="hist", bufs=2))
        el = ctx.enter_context(tc.tile_pool(name="elt", bufs=2))
        rp = ctx.enter_context(tc.tile_pool(name="rrh", bufs=4))
        pgr = ctx.enter_context(tc.tile_pool(name="psr", bufs=2, space="PSUM"))
        pgz = ctx.enter_context(tc.tile_pool(name="psz", bufs=2, space="PSUM"))
        pgl = ctx.enter_context(tc.tile_pool(name="psl", bufs=2, space="PSUM"))
        p3 = ctx.enter_context(tc.tile_pool(name="ps3", bufs=2, space="PSUM"))
        o3 = ctx.enter_context(tc.tile_pool(name="o3", bufs=3))
        bp = ctx.enter_context(tc.tile_pool(name="rb", bufs=1))

        # resident bf16 weights (shipped pre-cast from the host)
        wx_sb, wh_sb, bt = {}, {}, {}
        for g in "rzl":
            wx_sb[g] = wp.tile([128, KC * H], bf16, tag=f"wx{g}", name=f"wx{g}sb")
            wh_sb[g] = wp.tile([128, KC * H], bf16, tag=f"wh{g}", name=f"wh{g}sb")
            for kc in range(KC):
                nc.sync.dma_start(wx_sb[g][:, kc * H:(kc + 1) * H],
                                  wx[g][kc * 128:(kc + 1) * 128, :])
                nc.sync.dma_start(wh_sb[g][:, kc * H:(kc + 1) * H],
                                  wh[g][kc * 128:(kc + 1) * 128, :])
            bt[g] = bp.tile([128, JC], f32, tag=f"b{g}", name=f"bt{g}")
            for jc in range(JC):
                nc.sync.dma_start(bt[g][:, jc:jc + 1],
                                  bias[g][jc * 128:(jc + 1) * 128, :])
        wo_sb = wp.tile([128, KC * H], bf16, tag="wo")
        for kc in range(KC):
            nc.sync.dma_start(wo_sb[:, kc * H:(kc + 1) * H],
                              wo_d[kc * 128:(kc + 1) * 128, :])
        ident = wp.tile([128, 128], bf16, tag="ident")
        nc.sync.dma_start(ident[:], id_d[:])
        bo_t = bp.tile([128, JC], f32, tag="bo")
        for jc in range(JC):
            nc.sync.dma_start(bo_t[:, jc:jc + 1],
                              bo_d[jc * 128:(jc + 1) * 128, :])

        hz = bp.tile([128, CW], bf16, tag="h0")
        nc.vector.memset(hz[:], 0.0)

        # ---- X-projection machinery (in-stream "phase 1") ----
        xt_blk = {}         # bi -> [128, KC*CB] bf16 input slice
        X_blk = {}          # bi -> {g: [128, JC*CB] bf16}

        def xt_load(bi):
            t_ = xtp.tile([128, KC * CB], bf16, tag="xt")
            for kc in range(KC):
                nc.sync.dma_start(
                    t_[:, kc * CB:(kc + 1) * CB],
                    xT[kc * 128:(kc + 1) * 128, bi * CB:(bi + 1) * CB])
            xt_blk[bi] = t_

        def x_alloc(bi):
            X_blk[bi] = {g: xsb.tile([128, JC * CB], bf16, tag=f"X{g}",
                                     name=f"X{g}b")
                         for g in "rzl"}

        def x_unit(bi, g, jc):
            """One (gate, jc) X-projection unit for block bi (8 MMs N=128)."""
            ps = p3.tile([128, CB], f32, tag="pso")
            xt = xt_blk[bi]
            for kc in range(KC):
                nc.tensor.matmul(
                    ps[:],
                    lhsT=wx_sb[g][:, kc * H + jc * 128:kc * H + (jc + 1) * 128],
                    rhs=xt[:, kc * CB:(kc + 1) * CB],
                    start=(kc == 0), stop=(kc == KC - 1))
            nc.scalar.activation(X_blk[bi][g][:, jc * CB:(jc + 1) * CB],
                                 ps[:], AF.Identity, bias=bt[g][:, jc:jc + 1])

        UNITS = [(g, jc) for g in "rzl" for jc in range(JC)]   # 24 per block

        def wo_unit(hsrc, bi_out, jc):
            pso = p3.tile([128, CB], f32, tag="pso")
            for kc in range(KC):
                nc.tensor.matmul(
                    pso[:],
                    lhsT=wo_sb[:, (kc * JC + jc) * 128:(kc * JC + jc + 1) * 128],
                    rhs=hsrc[:, kc * CB:(kc + 1) * CB],
                    start=(kc == 0), stop=(kc == KC - 1))
            ou = o3.tile([128, CB], f32, tag="ou")
            nc.scalar.activation(ou[:], pso[:], AF.Relu, bias=bo_t[:, jc:jc + 1])
            nc.sync.dma_start(
                outT[:, (bi_out * JC + jc) * CB:(bi_out * JC + jc + 1) * CB],
                ou[:])

        def gate_mm(ps, wt, src_slices, xfold=None):
            for jc in range(JC):
                reg = ps[:, jc * BL:(jc + 1) * BL]
                if xfold is not None:
                    nc.tensor.matmul(reg, lhsT=ident[:], rhs=xfold[jc],
                                     start=True, stop=False)
                for kc in range(KC):
                    nc.tensor.matmul(
                        reg,
                        lhsT=wt[:, (kc * JC + jc) * 128:(kc * JC + jc + 1) * 128],
                        rhs=src_slices[kc],
                        start=(xfold is None and kc == 0),
                        stop=(kc == KC - 1))

        # prime: X projections for blocks 0 and 1 up-front
        for bi in range(min(2, nblk)):
            xt_load(bi)
            x_alloc(bi)
            for g, jc in UNITS:
                x_unit(bi, g, jc)

        hist_prev = None
        for bi in range(nblk):
            if bi + 2 < nblk:
                xt_load(bi + 2)
                x_alloc(bi + 2)
            # X-projection units for block bi+2, spread across this block's
            # steps: 2 units on steps 0-7, 1 unit on steps 8-15.
            queue = list(UNITS) if bi + 2 < nblk else []

            Xb = X_blk[bi]
            hist = hi.tile([128, KC * CB], bf16, tag="hist")
            hview = hist[:].rearrange("p (c t b) -> p c t b", c=KC, t=BLK)

            for dt in range(BLK):
                if bi == 0 and dt == 0:
                    hsl = [hz[:, kc * BL:(kc + 1) * BL] for kc in range(KC)]
                    hap = hz[:].rearrange("p (c b) -> p c b", c=KC)
                elif dt == 0:
                    hsl = [hist_prev[:, kc * CB + (BLK - 1) * BL:
                                     kc * CB + BLK * BL] for kc in range(KC)]
                    hap = (hist_prev[:]
                           .rearrange("p (c t b) -> p c t b", c=KC, t=BLK)
                           [:, :, BLK - 1, :])
                else:
                    hsl = [hist[:, kc * CB + (dt - 1) * BL:kc * CB + dt * BL]
                           for kc in range(KC)]
                    hap = hview[:, :, dt - 1, :]

                def xf(g):
                    return [Xb[g][:, jc * CB + dt * BL:jc * CB + (dt + 1) * BL]
                            for jc in range(JC)]

                def xap(g):
                    return (Xb[g][:]
                            .rearrange("p (c t b) -> p c t b", c=JC, t=BLK)
                            [:, :, dt, :])

                c3 = "p (c b) -> p c b"

                # r gate first, z gate second (measured faster than z-first:
                # the PE semaphore bump covering psr's stop lands mid-z-run,
                # waking sigmoid_r earlier).
                psr = pgr.tile([128, CW], f32, tag="gr")
                gate_mm(psr, wh_sb["r"], hsl, xfold=xf("r"))
                psz = pgz.tile([128, CW], f32, tag="gz")
                gate_mm(psz, wh_sb["z"], hsl)

                r = rp.tile([128, CW], f32, tag="r")
                nc.scalar.activation(r[:], psr[:], AF.Sigmoid)
                rh = rp.tile([128, CW], bf16, tag="rh")
                nc.vector.tensor_mul(rh[:].rearrange(c3, c=KC),
                                     r[:].rearrange(c3, c=KC), hap)
                # scheduler fence: keep rh's engine wait from being merged
                # with tz's (whose z-PSUM threshold would delay rh, and
                # with it the whole l gate, to the end of the z matmuls)
                tc.no_sync_barrier()

                # one X unit rides in the sigmoid_r/rh handoff window
                if queue:
                    g_, jc_ = queue.pop(0)
                    x_unit(bi + 2, g_, jc_)

                # l gate: X_l folded into PSUM, rhs = (r*h)^T
                psl = pgl.tile([128, CW], f32, tag="gl")
                gate_mm(psl, wh_sb["l"],
                        [rh[:, kc * BL:(kc + 1) * BL] for kc in range(KC)],
                        xfold=xf("l"))

                # z post-chain + w = (1-z)*h (hides under the l matmuls)
                tz = el.tile([128, CW], f32, tag="tz")
                nc.vector.tensor_add(tz[:].rearrange(c3, c=KC),
                                     psz[:].rearrange(c3, c=KC), xap("z"))
                z = el.tile([128, CW], f32, tag="z")
                nc.scalar.activation(z[:], tz[:], AF.Sigmoid)
                zm1 = el.tile([128, CW], f32, tag="zm1")
                nc.vector.tensor_scalar(zm1[:], z[:], -1.0, 1.0,
                                        ALU.mult, ALU.add)
                w = el.tile([128, CW], f32, tag="w")
                nc.vector.tensor_mul(w[:].rearrange(c3, c=KC),
                                     zm1[:].rearrange(c3, c=KC), hap)
                # fence: keep the z post-chain waits (z-PSUM / sigmoid_z
                # thresholds) from merging with the tail pair's tanh wait,
                # which would delay w past tanh and stretch the tail
                tc.no_sync_barrier()

                # tail window: alternate Wo chunk / X unit
                if dt % 2 == 0:
                    if bi > 0:
                        wo_unit(hist_prev, bi - 1, dt // 2)
                elif queue:
                    g_, jc_ = queue.pop(0)
                    x_unit(bi + 2, g_, jc_)

                # tail: hh = tanh(psl); h_new = w + z*hh -> hist (bf16)
                hh = el.tile([128, CW], f32, tag="hh")
                nc.scalar.activation(hh[:], psl[:], AF.Tanh)
                n = el.tile([128, CW], f32, tag="n")
                nc.vector.tensor_mul(n[:], z[:], hh[:])
                nc.vector.tensor_add(hview[:, :, dt, :],
                                     w[:].rearrange(c3, c=KC),
                                     n[:].rearrange(c3, c=KC))

            # issue any leftover units (only when nblk is tiny)
            for g_, jc_ in queue:
                x_unit(bi + 2, g_, jc_)
            X_blk.pop(bi, None)
            xt_blk.pop(bi, None)
            hist_prev = hist

        for jc in range(JC):
            wo_unit(hist_prev, nblk - 1, jc)

    nc.compile()
    return nc


def get_program(T):
    if T not in _CACHE:
        _CACHE[T] = build_program(T)
    return _CACHE[T]


def _bf(a):
    return np.ascontiguousarray(np.asarray(a, dtype=np.float32)).astype(
        ml_dtypes.bfloat16)


def make_inmaps(input, Wr, br, Wz, bz, Wl, bl, Wo, bo):
    Tt = input.shape[1]
    cols = BL * Tt
    w_common = {
        "wxr": _bf(Wr[H:]), "whr": _bf(Wr[:H]),
        "wxz": _bf(Wz[H:]), "whz": _bf(Wz[:H]),
        "wxl": _bf(Wl[H:]), "whl": _bf(Wl[:H]),
        "br": np.ascontiguousarray(br.reshape(H, 1)).astype(np.float32),
        "bz": np.ascontiguousarray(bz.reshape(H, 1)).astype(np.float32),
        "bl": np.ascontiguousarray(bl.reshape(H, 1)).astype(np.float32),
        "wo": _bf(Wo),
        "bo": np.ascontiguousarray(bo.reshape(H, 1)).astype(np.float32),
        "ident": np.eye(128, dtype=np.float32).astype(ml_dtypes.bfloat16),
    }
    in_maps = []
    for c in range(NCORES):
        xl = np.asarray(input[c * BL:(c + 1) * BL], dtype=np.float32)
        xTl = _bf(xl.transpose(2, 1, 0).reshape(H, cols))
        in_maps.append({"xT": xTl, **w_common})
    return in_maps


def kernel(input, Wr, br, Wz, bz, Wl, bl, Wo, bo):
    Tt = input.shape[1]
    prog = get_program(Tt)
    in_maps = make_inmaps(input, Wr, br, Wz, bz, Wl, bl, Wo, bo)
    res = bass_utils.run_bass_kernel_spmd(prog, in_maps,
                                          core_ids=list(range(NCORES)))
    nblk = Tt // BLK
    outs = []
    for c in range(NCORES):
        oT = res.results[c]["outT"]              # [128, nblk*JC*BLK*BL]
        o = oT.reshape(128, nblk, JC, BLK, BL)   # p, bi, j, dt, b
        o = o.transpose(4, 1, 3, 2, 0).reshape(BL, Tt, H)
        outs.append(o)
    return np.ascontiguousarray(np.concatenate(outs, axis=0))


# revision 20
# speedup vs baseline: 1.1931x; 1.0311x over previous
"""GRU kernel for Trainium2, 8 NeuronCores, data-parallel over batch.

Strategy (v4)
-------------
reference:  per step t (T=512):
    gi = [h, x_t]; r = sig(gi@Wr+br); z = sig(gi@Wz+bz)
    hh = tanh([h*r, x_t]@Wl+bl); h = (1-z)h + z*hh; out_t = relu(h@Wo+bo)

Per core (B_local=8 rows, fully transposed domain; state h^T is bf16
[128 part, kc-major 8 chunks x 8 batch cols]):

  All weights ship from the host pre-cast to bf16 and stay resident in
  SBUF.  x^T ships as bf16 and streams per 16-step block.

  The x projections X_g = Wx_g^T x^T + b_g are NOT a separate phase:
  they are computed in-stream, two blocks ahead of the recurrence,
  into an SBUF ring — issued one (gate, jc) unit at a time inside the
  two PE idle windows of each step (the sigmoid_r/rh handoff window
  and the tanh/h-update tail).  This removes the serial up-front phase
  and the DRAM X round-trip entirely.

  Per recurrence step:
    - z gate, then r gate (contiguous PE run).  X_r is folded into
      PSUM via an identity-stationary matmul; sigmoid_r reads PSUM.
    - l gate with X_l identity-folded, rhs = (r*h)^T.
    - update: w = (1-z)*h precomputed during the l window;
      h_new = w + z*tanh(psl) written straight into the chunk-major
      bf16 history tile.
    - idle windows carry: one X-projection unit each, plus one jc
      chunk of the previous block's output projection relu(Wo h + bo)
      every other step.
"""
import numpy as np
import ml_dtypes
from contextlib import ExitStack

import concourse.bass as bass
import concourse.tile as tile
from concourse import bacc, mybir
from concourse import bass_utils

B, T_FULL, D, H = 64, 512, 1024, 1024
NCORES = 8
BL = B // NCORES            # 8 batch rows per core
KC = H // 128               # 8 contraction chunks
JC = H // 128               # 8 output chunks
BLK = 16                    # recurrence steps per output-projection block
CB = BLK * BL               # 128 activation columns per block

f32 = mybir.dt.float32
bf16 = mybir.dt.bfloat16
AF = mybir.ActivationFunctionType
ALU = mybir.AluOpType

_CACHE = {}


def build_program(T):
    cols = BL * T
    nblk = T // BLK
    assert T % BLK == 0
    CW = BL * KC            # 64: cols of a state tile (kc-major, b minor)

    nc = bacc.Bacc("TRN2", target_bir_lowering=False, debug=False, num_devices=1)

    xT = nc.dram_tensor("xT", (H, cols), bf16, kind="ExternalInput").ap()
    wx = {g: nc.dram_tensor(f"wx{g}", (D, H), bf16, kind="ExternalInput").ap()
          for g in "rzl"}
    wh = {g: nc.dram_tensor(f"wh{g}", (H, H), bf16, kind="ExternalInput").ap()
          for g in "rzl"}
    bias = {g: nc.dram_tensor(f"b{g}", (H, 1), f32, kind="ExternalInput").ap()
            for g in "rzl"}
    wo_d = nc.dram_tensor("wo", (H, H), bf16, kind="ExternalInput").ap()
    bo_d = nc.dram_tensor("bo", (H, 1), f32, kind="ExternalInput").ap()
    id_d = nc.dram_tensor("ident", (128, 128), bf16, kind="ExternalInput").ap()
    outT = nc.dram_tensor("outT", (128, nblk * JC * CB), f32,
                          kind="ExternalOutput").ap()

    with tile.TileContext(nc) as tc, ExitStack() as ctx:
        wp = ctx.enter_context(tc.tile_pool(name="rw", bufs=1))
        xtp = ctx.enter_context(tc.tile_pool(name="xtp", bufs=3))
        xsb = ctx.enter_context(tc.tile_pool(name="xsb", bufs=3))
        hi = ctx.enter_context(tc.tile_pool(name="hist", bufs=2))
        el = ctx.enter_context(tc.tile_pool(name="elt", bufs=2))
        rp = ctx.enter_context(tc.tile_pool(name="rrh", bufs=4))
        pgr = ctx.enter_context(tc.tile_pool(name="psr", bufs=2, space="PSUM"))
        pgz = ctx.enter_context(tc.tile_pool(name="psz", bufs=2, space="PSUM"))
        pgl = ctx.enter_context(tc.tile_pool(name="psl", bufs=2, space="PSUM"))
        p3 = ctx.enter_context(tc.tile_pool(name="ps3", bufs=2, space="PSUM"))
        o3 = ctx.enter_context(tc.tile_pool(name="o3", bufs=3))
        bp = ctx.enter_context(tc.tile_pool(name="rb", bufs=1))

        # resident bf16 weights (shipped pre-cast from the host)
        wx_sb, wh_sb, bt = {}, {}, {}
        for g in "rzl":
            wx_sb[g] = wp.tile([128, KC * H], bf16, tag=f"wx{g}", name=f"wx{g}sb")
            wh_sb[g] = wp.tile([128, KC * H], bf16, tag=f"wh{g}", name=f"wh{g}sb")
            for kc in range(KC):
                nc.sync.dma_start(wx_sb[g][:, kc * H:(kc + 1) * H],
                                  wx[g][kc * 128:(kc + 1) * 128, :])
                nc.sync.dma_start(wh_sb[g][:, kc * H:(kc + 1) * H],
                                  wh[g][kc * 128:(kc + 1) * 128, :])
            bt[g] = bp.tile([128, JC], f32, tag=f"b{g}", name=f"bt{g}")
            for jc in range(JC):
                nc.sync.dma_start(bt[g][:, jc:jc + 1],
                                  bias[g][jc * 128:(jc + 1) * 128, :])
        wo_sb = wp.tile([128, KC * H], bf16, tag="wo")
        for kc in range(KC):
            nc.sync.dma_start(wo_sb[:, kc * H:(kc + 1) * H],
                              wo_d[kc * 128:(kc + 1) * 128, :])
        ident = wp.tile([128, 128], bf16, tag="ident")
        nc.sync.dma_start(ident[:], id_d[:])
        bo_t = bp.tile([128, JC], f32, tag="bo")
        for jc in range(JC):
            nc.sync.dma_start(bo_t[:, jc:jc + 1],
                              bo_d[jc * 128:(jc + 1) * 128, :])

        hz = bp.tile([128, CW], bf16, tag="h0")
        nc.vector.memset(hz[:], 0.0)

        # ---- X-projection machinery (in-stream "phase 1") ----
        xt_blk = {}         # bi -> [128, KC*CB] bf16 input slice
        X_blk = {}          # bi -> {g: [128, JC*CB] bf16}

        def xt_load(bi):
            t_ = xtp.tile([128, KC * CB], bf16, tag="xt")
            for kc in range(KC):
                nc.sync.dma_start(
                    t_[:, kc * CB:(kc + 1) * CB],
                    xT[kc * 128:(kc + 1) * 128, bi * CB:(bi + 1) * CB])
            xt_blk[bi] = t_

        def x_alloc(bi):
            X_blk[bi] = {g: xsb.tile([128, JC * CB], bf16, tag=f"X{g}",
                                     name=f"X{g}b")
                         for g in "rzl"}

        def x_unit(bi, g, jc):
            """One (gate, jc) X-projection unit for block bi (8 MMs N=128)."""
            ps = p3.tile([128, CB], f32, tag="pso")
            xt = xt_blk[bi]
            for kc in range(KC):
                nc.tensor.matmul(
                    ps[:],
                    lhsT=wx_sb[g][:, kc * H + jc * 128:kc * H + (jc + 1) * 128],
                    rhs=xt[:, kc * CB:(kc + 1) * CB],
                    start=(kc == 0), stop=(kc == KC - 1))
            nc.scalar.activation(X_blk[bi][g][:, jc * CB:(jc + 1) * CB],
                                 ps[:], AF.Identity, bias=bt[g][:, jc:jc + 1])

        UNITS = [(g, jc) for g in "rzl" for jc in range(JC)]   # 24 per block

        def wo_unit(hsrc, bi_out, jc):
            pso = p3.tile([128, CB], f32, tag="pso")
            for kc in range(KC):
                nc.tensor.matmul(
                    pso[:],
                    lhsT=wo_sb[:, (kc * JC + jc) * 128:(kc * JC + jc + 1) * 128],
                    rhs=hsrc[:, kc * CB:(kc + 1) * CB],
                    start=(kc == 0), stop=(kc == KC - 1))
            ou = o3.tile([128, CB], f32, tag="ou")
            nc.scalar.activation(ou[:], pso[:], AF.Relu, bias=bo_t[:, jc:jc + 1])
            nc.sync.dma_start(
                outT[:, (bi_out * JC + jc) * CB:(bi_out * JC + jc + 1) * CB],
                ou[:])

        def gate_mm(ps, wt, src_slices, xfold=None):
            for jc in range(JC):
                reg = ps[:, jc * BL:(jc + 1) * BL]
                if xfold is not None:
                    nc.tensor.matmul(reg, lhsT=ident[:], rhs=xfold[jc],
                                     start=True, stop=False)
                for kc in range(KC):
                    nc.tensor.matmul(
                        reg,
                        lhsT=wt[:, (kc * JC + jc) * 128:(kc * JC + jc + 1) * 128],
                        rhs=src_slices[kc],
                        start=(xfold is None and kc == 0),
                        stop=(kc == KC - 1))

        # prime: X projections for blocks 0 and 1 up-front
        for bi in range(min(2, nblk)):
            xt_load(bi)
            x_alloc(bi)
            for g, jc in UNITS:
                x_unit(bi, g, jc)

        hist_prev = None
        for bi in range(nblk):
            if bi + 2 < nblk:
                xt_load(bi + 2)
                x_alloc(bi + 2)
            # X-projection units for block bi+2, spread across this block's
            # steps: 2 units on steps 0-7, 1 unit on steps 8-15.
            queue = list(UNITS) if bi + 2 < nblk else []

            Xb = X_blk[bi]
            hist = hi.tile([128, KC * CB], bf16, tag="hist")
            hview = hist[:].rearrange("p (c t b) -> p c t b", c=KC, t=BLK)

            for dt in range(BLK):
                if bi == 0 and dt == 0:
                    hsl = [hz[:, kc * BL:(kc + 1) * BL] for kc in range(KC)]
                    hap = hz[:].rearrange("p (c b) -> p c b", c=KC)
                elif dt == 0:
                    hsl = [hist_prev[:, kc * CB + (BLK - 1) * BL:
                                     kc * CB + BLK * BL] for kc in range(KC)]
                    hap = (hist_prev[:]
                           .rearrange("p (c t b) -> p c t b", c=KC, t=BLK)
                           [:, :, BLK - 1, :])
                else:
                    hsl = [hist[:, kc * CB + (dt - 1) * BL:kc * CB + dt * BL]
                           for kc in range(KC)]
                    hap = hview[:, :, dt - 1, :]

                def xf(g):
                    return [Xb[g][:, jc * CB + dt * BL:jc * CB + (dt + 1) * BL]
                            for jc in range(JC)]

                def xap(g):
                    return (Xb[g][:]
                            .rearrange("p (c t b) -> p c t b", c=JC, t=BLK)
                            [:, :, dt, :])

                c3 = "p (c b) -> p c b"

                # r gate first, z gate second (measured faster than z-first:
                # the PE semaphore bump covering psr's stop lands mid-z-run,
                # waking sigmoid_r earlier).
                psr = pgr.tile([128, CW], f32, tag="gr")
                gate_mm(psr, wh_sb["r"], hsl, xfold=xf("r"))
                psz = pgz.tile([128, CW], f32, tag="gz")
                gate_mm(psz, wh_sb["z"], hsl)

                r = rp.tile([128, CW], f32, tag="r")
                nc.scalar.activation(r[:], psr[:], AF.Sigmoid)
                rh = rp.tile([128, CW], bf16, tag="rh")
                nc.vector.tensor_mul(rh[:].rearrange(c3, c=KC),
                                     r[:].rearrange(c3, c=KC), hap)
                # scheduler fence: keep rh's engine wait from being merged
                # with tz's (whose z-PSUM threshold would delay rh, and
                # with it the whole l gate, to the end of the z matmuls)
                tc.no_sync_barrier()

                # l gate: X_l folded into PSUM, rhs = (r*h)^T
                psl = pgl.tile([128, CW], f32, tag="gl")
                gate_mm(psl, wh_sb["l"],
                        [rh[:, kc * BL:(kc + 1) * BL] for kc in range(KC)],
                        xfold=xf("l"))

                # z post-chain + w = (1-z)*h (hides under the l matmuls)
                tz = el.tile([128, CW], f32, tag="tz")
                nc.vector.tensor_add(tz[:].rearrange(c3, c=KC),
                                     psz[:].rearrange(c3, c=KC), xap("z"))
                z = el.tile([128, CW], f32, tag="z")
                nc.scalar.activation(z[:], tz[:], AF.Sigmoid)
                zm1 = el.tile([128, CW], f32, tag="zm1")
                nc.vector.tensor_scalar(zm1[:], z[:], -1.0, 1.0,
                                        ALU.mult, ALU.add)
                w = el.tile([128, CW], f32, tag="w")
                nc.vector.tensor_mul(w[:].rearrange(c3, c=KC),
                                     zm1[:].rearrange(c3, c=KC), hap)
                # fence: keep the z post-chain waits (z-PSUM / sigmoid_z
                # thresholds) from merging with the tail pair's tanh wait,
                # which would delay w past tanh and stretch the tail
                tc.no_sync_barrier()

                # tail: hh = tanh(psl); h_new = w + z*hh -> hist (bf16).
                # Issued BEFORE any fill work so tanh is first in the ACT
                # queue after sigmoid_z (a relu/X-ACT in front would stall
                # the whole tail behind its later PSUM dependency).
                hh = el.tile([128, CW], f32, tag="hh")
                nc.scalar.activation(hh[:], psl[:], AF.Tanh)
                n = el.tile([128, CW], f32, tag="n")
                nc.vector.tensor_mul(n[:], z[:], hh[:])
                nc.vector.tensor_add(hview[:, :, dt, :],
                                     w[:].rearrange(c3, c=KC),
                                     n[:].rearrange(c3, c=KC))

                # fill the tanh/h-update PE idle window: Wo chunk on even
                # steps plus X units (2 on odd steps) -> 24 units/block
                if dt % 2 == 0:
                    if bi > 0:
                        wo_unit(hist_prev, bi - 1, dt // 2)
                    if queue:
                        g_, jc_ = queue.pop(0)
                        x_unit(bi + 2, g_, jc_)
                else:
                    for _ in range(2):
                        if queue:
                            g_, jc_ = queue.pop(0)
                            x_unit(bi + 2, g_, jc_)

            # issue any leftover units (only when nblk is tiny)
            for g_, jc_ in queue:
                x_unit(bi + 2, g_, jc_)
            X_blk.pop(bi, None)
            xt_blk.pop(bi, None)
            hist_prev = hist

        for jc in range(JC):
            wo_unit(hist_prev, nblk - 1, jc)

    nc.compile()
    return nc


def get_program(T):
    if T not in _CACHE:
        _CACHE[T] = build_program(T)
    return _CACHE[T]


def _bf(a):
    return np.ascontiguousarray(np.asarray(a, dtype=np.float32)).astype(
        ml_dtypes.bfloat16)


def make_inmaps(input, Wr, br, Wz, bz, Wl, bl, Wo, bo):
    Tt = input.shape[1]
    cols = BL * Tt
    w_common = {
        "wxr": _bf(Wr[H:]), "whr": _bf(Wr[:H]),
        "wxz": _bf(Wz[H:]), "whz": _bf(Wz[:H]),
        "wxl": _bf(Wl[H:]), "whl": _bf(Wl[:H]),
        "br": np.ascontiguousarray(br.reshape(H, 1)).astype(np.float32),
        "bz": np.ascontiguousarray(bz.reshape(H, 1)).astype(np.float32),
        "bl": np.ascontiguousarray(bl.reshape(H, 1)).astype(np.float32),
        "wo": _bf(Wo),
        "bo": np.ascontiguousarray(bo.reshape(H, 1)).astype(np.float32),
        "ident": np.eye(128, dtype=np.float32).astype(ml_dtypes.bfloat16),
    }
    in_maps = []
    for c in range(NCORES):
        xl = np.asarray(input[c * BL:(c + 1) * BL], dtype=np.float32)
        xTl = _bf(xl.transpose(2, 1, 0).reshape(H, cols))
        in_maps.append({"xT": xTl, **w_common})
    return in_maps


def kernel(input, Wr, br, Wz, bz, Wl, bl, Wo, bo):
    Tt = input.shape[1]
    prog = get_program(Tt)
    in_maps = make_inmaps(input, Wr, br, Wz, bz, Wl, bl, Wo, bo)
    res = bass_utils.run_bass_kernel_spmd(prog, in_maps,
                                          core_ids=list(range(NCORES)))
    nblk = Tt // BLK
    outs = []
    for c in range(NCORES):
        oT = res.results[c]["outT"]              # [128, nblk*JC*BLK*BL]
        o = oT.reshape(128, nblk, JC, BLK, BL)   # p, bi, j, dt, b
        o = o.transpose(4, 1, 3, 2, 0).reshape(BL, Tt, H)
        outs.append(o)
    return np.ascontiguousarray(np.concatenate(outs, axis=0))
